# revision 22
# baseline (speedup 1.0000x reference)
"""AttentionBlock (GroupNorm -> MHA -> out-proj -> residual) on 8 TRN2 NeuronCores.

Problem: x (16, 512, 32, 32) fp32; GroupNorm(groups=1) over (C,H,W); spatial
flattened to a 1024-token sequence; 4 heads x 128 dim self-attention; output
projection; residual add.

Sharding: pure data-parallel over batch - 2 batch elements per core, no
collectives. Each core runs the identical program on its own x shard.

Layout strategy (per batch element, everything channel-major [c, s] so the PE
contracts over partitions with zero transposes):
  - GroupNorm stats via ACT Square+accum / DVE row-reduce, cross-partition via
    a ones-vector matmul; rstd by Newton rsqrt on DVE (avoids ACT table
    switch away from the Exp set). Partition broadcasts are 0-stride-AP DMAs.
  - QKV: qkv_cs[m, s] = qkv_wT.T @ x_norm  (Q, K stay [c, s]); V is computed
    directly transposed, vT[s, vd] = x_norm.T @ qkv_wT_v, so attention needs
    no transposes at all.
  - scoresT[s2, s1] = K.T @ Q per head; exp on ACT (PSUM -> SBUF, fused
    1/sqrt(hd) scale; softmax max-subtraction skipped - scores are O(1) by
    construction so exp cannot overflow).
  - row sums of exp via ones-vector matmul (PSUM accumulation over s2 tiles);
    normalization applied to the (small) AV output, with the reciprocal
    broadcast across partitions by GPSIMD.
  - out-proj + residual fused into the PSUM-evacuation op on DVE.
  - All matmuls run in float32r (rounded-fp32, full PE rate at N=512).

GroupNorm's affine (gn_weight/gn_bias) is folded into the QKV weights on the
host: qkv = W @ (xn * g + b) = (W * g) @ xn + (qkv_b + W @ b).
"""
import sys

sys.path.insert(0, "/opt/trn_rl_repo")

import numpy as np

import concourse.bass as bass
import concourse.mybir as mybir
import concourse.tile as tile
from concourse import bacc
from concourse.bass_utils import run_bass_kernel_spmd

F32 = mybir.dt.float32
F32R = mybir.dt.float32r
AX = mybir.AxisListType
OP = mybir.AluOpType
ACT = mybir.ActivationFunctionType

N_CORES = 8
B, C, H, W = 16, 512, 32, 32
S = H * W                     # 1024 sequence positions
NH, HD = 4, C // 4            # 4 heads x 128
BPC = B // N_CORES            # 2 batch elements per core
CT = C // 128                 # 4 channel tiles
ST = S // 128                 # 8 sequence tiles
NCH = S // 512                # 2 free-dim chunks of 512
EPS = 1e-5
SCALE = 1.0 / float(np.sqrt(HD))
N_ELEM = float(C * S)


DEFAULT_CFG = {
    "sc_bufs": 2, "av_bufs": 2, "mm_bufs": 3, "et_bufs": 12,
    "xn_bufs": 4, "qk_bufs": 8, "vt_bufs": 8, "on_bufs": 4,
    "xload_bufs": 5, "res_bufs": 2, "rx_bufs": 2,
    # debug/timing-shape flags (change semantics; model experiments only)
    "skip_norm": False, "exp_on_dve": False,
}


def build_program(use_v_bias: bool, cfg: dict | None = None) -> bass.Bass:
    cfg = {**DEFAULT_CFG, **(cfg or {})}
    nc = bacc.Bacc()
    x_d = nc.dram_tensor("x", [BPC, C, S], F32, kind="ExternalInput")
    wqkv_d = nc.dram_tensor("wqkvT", [C, 3 * C], F32, kind="ExternalInput")
    bqkv_d = nc.dram_tensor("bqkv", [3 * C], F32, kind="ExternalInput")
    wout_d = nc.dram_tensor("woutT", [C, C], F32, kind="ExternalInput")
    bout_d = nc.dram_tensor("bout", [C], F32, kind="ExternalInput")
    y_d = nc.dram_tensor("y", [BPC, C, S], F32, kind="ExternalOutput")
    # DRAM scratch for partition broadcasts (SBUF->DRAM->0-stride-read-back)
    scr_ms = nc.dram_tensor("scr_ms", [BPC, 2], F32)
    scr_rcp = nc.dram_tensor("scr_rcp", [BPC, NH, NCH, 512], F32)

    with tile.TileContext(nc) as tc:
        with (
            tc.tile_pool(name="const", bufs=1) as cpool,
            tc.tile_pool(name="sb", bufs=1) as sb,
            tc.tile_pool(name="ps", bufs=1, space="PSUM") as ps,
        ):
            # ---- constants ----
            wq = []
            for k in range(CT):
                t = cpool.tile([128, 3 * C], F32R, name=f"wq{k}")
                nc.gpsimd.dma_start(out=t, in_=wqkv_d[k * 128:(k + 1) * 128, :])
                wq.append(t)
            wo = []
            for k in range(CT):
                t = cpool.tile([128, C], F32R, name=f"wo{k}")
                nc.gpsimd.dma_start(out=t, in_=wout_d[k * 128:(k + 1) * 128, :])
                wo.append(t)
            bqkv_t = cpool.tile([128, 12], F32, name="bqkv_t")
            nc.sync.dma_start(out=bqkv_t, in_=bqkv_d[:].rearrange("(m p) -> p m", p=128))
            bout_t = cpool.tile([128, CT], F32, name="bout_t")
            nc.sync.dma_start(out=bout_t, in_=bout_d[:].rearrange("(m p) -> p m", p=128))
            ones32 = cpool.tile([128, 1], F32, name="ones32")
            nc.vector.memset(ones32, 1.0)
            ones_t = cpool.tile([128, 1], F32R, name="ones_t")
            nc.vector.tensor_copy(out=ones_t, in_=ones32)
            if use_v_bias:
                bv_bc = cpool.tile([128, C], F32, name="bv_bc")
                nc.sync.dma_start(
                    out=bv_bc,
                    in_=bqkv_d[2 * C:3 * C].rearrange("(o s) -> o s", o=1)
                    .partition_broadcast(128))

            for b in range(BPC):
                # ---- GroupNorm statistics ----
                xts = []
                partials = sb.tile([128, 2 * CT], F32, tag="part", bufs=2, name=f"part{b}")
                for t in range(CT):
                    xt = sb.tile([128, S], F32, tag="xload", bufs=cfg["xload_bufs"], name=f"x{b}_{t}")
                    nc.sync.dma_start(out=xt, in_=x_d[b, t * 128:(t + 1) * 128, :])
                    xts.append(xt)
                    sq = sb.tile([128, S], F32, tag="sqscr", bufs=2, name=f"sq{b}_{t}")
                    nc.scalar.activation(out=sq, in_=xt, func=ACT.Square,
                                         accum_out=partials[:, CT + t:CT + t + 1])
                    nc.vector.reduce_sum(out=partials[:, t:t + 1], in_=xt, axis=AX.X)
                partials_r = sb.tile([128, 2 * CT], F32R, tag="partr", bufs=2,
                                     name=f"partr{b}")
                nc.vector.tensor_copy(out=partials_r, in_=partials)
                stat_ps = ps.tile([1, 512], F32, tag="row", bufs=1, name=f"stat{b}")
                nc.tensor.matmul(stat_ps[0:1, 0:2 * CT], ones_t, partials_r,
                                 start=True, stop=True)
                # scalar chain on partition 0; cols: 0=mean 1=y(rstd) 2=v 3,4=tmp
                scal = sb.tile([1, 5], F32, tag="scal", bufs=2, name=f"scal{b}")
                nc.vector.reduce_sum(out=scal[:, 3:4], in_=stat_ps[0:1, 0:CT], axis=AX.X)
                nc.vector.reduce_sum(out=scal[:, 4:5], in_=stat_ps[0:1, CT:2 * CT], axis=AX.X)
                nc.vector.tensor_scalar_mul(scal[:, 0:1], scal[:, 3:4], 1.0 / N_ELEM)
                nc.vector.tensor_scalar_mul(scal[:, 4:5], scal[:, 4:5], 1.0 / N_ELEM)
                nc.vector.tensor_tensor(out=scal[:, 3:4], in0=scal[:, 0:1],
                                        in1=scal[:, 0:1], op=OP.mult)
                nc.vector.tensor_tensor(out=scal[:, 2:3], in0=scal[:, 4:5],
                                        in1=scal[:, 3:4], op=OP.subtract)
                nc.vector.tensor_scalar_add(scal[:, 2:3], scal[:, 2:3], EPS)
                # Newton rsqrt: y0 = 1/v, y <- y*(1.5 - 0.5*v*y^2), 3 iters
                nc.vector.reciprocal(out=scal[:, 1:2], in_=scal[:, 2:3])
                for _ in range(3):
                    nc.vector.tensor_tensor(out=scal[:, 3:4], in0=scal[:, 1:2],
                                            in1=scal[:, 1:2], op=OP.mult)
                    nc.vector.tensor_tensor(out=scal[:, 3:4], in0=scal[:, 3:4],
                                            in1=scal[:, 2:3], op=OP.mult)
                    nc.vector.tensor_scalar(scal[:, 3:4], scal[:, 3:4], -0.5, 1.5,
                                            op0=OP.mult, op1=OP.add)
                    nc.vector.tensor_tensor(out=scal[:, 1:2], in0=scal[:, 1:2],
                                            in1=scal[:, 3:4], op=OP.mult)
                # broadcast (mean, rstd) to all partitions via a DRAM bounce
                nc.sync.dma_start(out=scr_ms[b], in_=scal[0:1, 0:2])
                mbc = sb.tile([128, 2], F32, tag="mbc", bufs=2, name=f"mbc{b}")
                nc.sync.dma_start(
                    out=mbc,
                    in_=scr_ms[b].rearrange("(o s) -> o s", o=1).partition_broadcast(128))

                # ---- x_norm = (x - mean) * rstd, written as float32r ----
                xns = []
                for t in range(CT):
                    xn = sb.tile([128, S], F32R, tag="xn", bufs=cfg["xn_bufs"], name=f"xn{b}_{t}")
                    nc.vector.tensor_scalar(xn, xts[t], mbc[:, 0:1], mbc[:, 1:2],
                                            op0=OP.subtract, op1=OP.mult)
                    xns.append(xn)

                # ---- QKV projections: Q,K channel-major; V sequence-major ----
                qk = {}
                for m in (0, 4, 1, 5, 2, 6, 3, 7):
                    qt = sb.tile([128, S], F32R, tag="qk", bufs=cfg["qk_bufs"], name=f"qk{b}_{m}")
                    for ch in range(NCH):
                        mm = ps.tile([128, 512], F32, tag="mm", bufs=cfg["mm_bufs"], name=f"mmq{b}_{m}_{ch}")
                        for k in range(CT):
                            nc.tensor.matmul(mm, wq[k][:, m * 128:(m + 1) * 128],
                                             xns[k][:, ch * 512:(ch + 1) * 512],
                                             start=(k == 0), stop=(k == CT - 1))
                        nc.vector.tensor_scalar_add(qt[:, ch * 512:(ch + 1) * 512], mm,
                                                    bqkv_t[:, m:m + 1])
                    qk[m] = qt
                vts = []
                for st in range(ST):
                    vt = sb.tile([128, C], F32R, tag="vt", bufs=cfg["vt_bufs"], name=f"vt{b}_{st}")
                    mm = ps.tile([128, 512], F32, tag="mm", bufs=cfg["mm_bufs"], name=f"mmv{b}_{st}")
                    for k in range(CT):
                        nc.tensor.matmul(mm, xns[k][:, st * 128:(st + 1) * 128],
                                         wq[k][:, 2 * C:3 * C],
                                         start=(k == 0), stop=(k == CT - 1))
                    if use_v_bias:
                        nc.vector.scalar_tensor_tensor(out=vt, in0=mm, scalar=0.0,
                                                       in1=bv_bc, op0=OP.add, op1=OP.add)
                    else:
                        nc.vector.tensor_copy(out=vt, in_=mm)
                    vts.append(vt)

                # ---- attention, head by head ----
                on = []
                for h in range(NH):
                    ot = sb.tile([128, S], F32R, tag="on", bufs=cfg["on_bufs"], name=f"on{b}_{h}")
                    on.append(ot)
                for h in range(NH):
                    q_t, k_t = qk[h], qk[NH + h]
                    for ch in range(NCH):
                        ets = []
                        for st in range(ST):
                            sc = ps.tile([128, 512], F32, tag="sc", bufs=cfg["sc_bufs"],
                                         name=f"sc{b}_{h}_{ch}_{st}")
                            nc.tensor.matmul(sc, k_t[:, st * 128:(st + 1) * 128],
                                             q_t[:, ch * 512:(ch + 1) * 512],
                                             start=True, stop=True)
                            et = sb.tile([128, 512], F32R, tag="et", bufs=cfg["et_bufs"],
                                         name=f"et{b}_{h}_{ch}_{st}")
                            if cfg["exp_on_dve"]:
                                nc.vector.tensor_copy(out=et, in_=sc)
                            else:
                                nc.scalar.activation(out=et, in_=sc, func=ACT.Exp, scale=SCALE)
                            ets.append(et)
                        if not cfg["skip_norm"]:
                            row = ps.tile([1, 512], F32, tag="row", bufs=1,
                                          name=f"row{b}_{h}_{ch}")
                            for st in range(ST):
                                nc.tensor.matmul(row, ones_t, ets[st],
                                                 start=(st == 0), stop=(st == ST - 1))
                            rcp = sb.tile([1, 512], F32, tag="rcp", bufs=2,
                                          name=f"rcp{b}_{h}_{ch}")
                            nc.vector.reciprocal(out=rcp, in_=row)
                            nc.sync.dma_start(out=scr_rcp[b, h, ch], in_=rcp)
                            rbc = sb.tile([128, 512], F32, tag="rbc", bufs=2,
                                          name=f"rbc{b}_{h}_{ch}")
                            nc.sync.dma_start(
                                out=rbc,
                                in_=scr_rcp[b, h, ch].rearrange("(o s) -> o s", o=1)
                                .partition_broadcast(128))
                        av = ps.tile([128, 512], F32, tag="av", bufs=cfg["av_bufs"],
                                     name=f"av{b}_{h}_{ch}")
                        for st in range(ST):
                            nc.tensor.matmul(av, vts[st][:, h * HD:(h + 1) * HD], ets[st],
                                             start=(st == 0), stop=(st == ST - 1))
                        if cfg["skip_norm"]:
                            nc.vector.tensor_copy(
                                out=on[h][:, ch * 512:(ch + 1) * 512], in_=av)
                        else:
                            nc.vector.tensor_tensor(out=on[h][:, ch * 512:(ch + 1) * 512],
                                                    in0=av, in1=rbc, op=OP.mult)

                # ---- output projection + residual ----
                for m in range(CT):
                    rx = sb.tile([128, S], F32, tag="rx", bufs=cfg["rx_bufs"], name=f"rx{b}_{m}")
                    nc.sync.dma_start(out=rx, in_=x_d[b, m * 128:(m + 1) * 128, :])
                    res = sb.tile([128, S], F32, tag="res", bufs=cfg["res_bufs"], name=f"res{b}_{m}")
                    for ch in range(NCH):
                        mm = ps.tile([128, 512], F32, tag="mm", bufs=cfg["mm_bufs"],
                                     name=f"mmo{b}_{m}_{ch}")
                        for k in range(CT):
                            nc.tensor.matmul(mm, wo[k][:, m * 128:(m + 1) * 128],
                                             on[k][:, ch * 512:(ch + 1) * 512],
                                             start=(k == 0), stop=(k == CT - 1))
                        nc.vector.scalar_tensor_tensor(
                            out=res[:, ch * 512:(ch + 1) * 512], in0=mm,
                            scalar=bout_t[:, m:m + 1],
                            in1=rx[:, ch * 512:(ch + 1) * 512],
                            op0=OP.add, op1=OP.add)
                    nc.sync.dma_start(out=y_d[b, m * 128:(m + 1) * 128, :], in_=res)
    nc.finalize()
    return nc


def build_program_v2(use_v_bias: bool, cfg: dict | None = None) -> bass.Bass:
    """Phased emission: stats(b1) overlaps QKV(b0) (ACT is idle there), QKV(b1)
    fills PE gaps of attention(b0), and exp runs on [128, 1024] PSUM reads
    (halves ACT per-instr overhead). PSUM banks: sc 1x2 + av 2 + mm 3 + row 1 = 8.
    """
    cfg = {**DEFAULT_CFG, "xn_bufs": 8, "et_bufs": 8, "res_bufs": 1,
           "sqscr_bufs": 1, "xload_bufs": 4, "rx_bufs": 1, **(cfg or {})}
    nc = bacc.Bacc()
    x_d = nc.dram_tensor("x", [BPC, C, S], F32, kind="ExternalInput")
    wqkv_d = nc.dram_tensor("wqkvT", [C, 3 * C], F32, kind="ExternalInput")
    bqkv_d = nc.dram_tensor("bqkv", [3 * C], F32, kind="ExternalInput")
    wout_d = nc.dram_tensor("woutT", [C, C], F32, kind="ExternalInput")
    bout_d = nc.dram_tensor("bout", [C], F32, kind="ExternalInput")
    y_d = nc.dram_tensor("y", [BPC, C, S], F32, kind="ExternalOutput")
    scr_ms = nc.dram_tensor("scr_ms", [BPC, 2], F32)
    scr_rcp = nc.dram_tensor("scr_rcp", [BPC, NH, NCH, 512], F32)

    with tile.TileContext(nc) as tc:
        with (
            tc.tile_pool(name="const", bufs=1) as cpool,
            tc.tile_pool(name="sb", bufs=1) as sb,
            tc.tile_pool(name="ps", bufs=1, space="PSUM") as ps,
        ):
            wq = []
            for k in range(CT):
                t = cpool.tile([128, 3 * C], F32R, name=f"wq{k}")
                nc.gpsimd.dma_start(out=t, in_=wqkv_d[k * 128:(k + 1) * 128, :])
                wq.append(t)
            wo = []
            for k in range(CT):
                t = cpool.tile([128, C], F32R, name=f"wo{k}")
                nc.gpsimd.dma_start(out=t, in_=wout_d[k * 128:(k + 1) * 128, :])
                wo.append(t)
            bqkv_t = cpool.tile([128, 12], F32, name="bqkv_t")
            nc.sync.dma_start(out=bqkv_t, in_=bqkv_d[:].rearrange("(m p) -> p m", p=128))
            bout_t = cpool.tile([128, CT], F32, name="bout_t")
            nc.sync.dma_start(out=bout_t, in_=bout_d[:].rearrange("(m p) -> p m", p=128))
            ones32 = cpool.tile([128, 1], F32, name="ones32")
            nc.vector.memset(ones32, 1.0)
            ones_t = cpool.tile([128, 1], F32R, name="ones_t")
            nc.vector.tensor_copy(out=ones_t, in_=ones32)
            if use_v_bias:
                bv_bc = cpool.tile([128, C], F32, name="bv_bc")
                nc.sync.dma_start(
                    out=bv_bc,
                    in_=bqkv_d[2 * C:3 * C].rearrange("(o s) -> o s", o=1)
                    .partition_broadcast(128))

            def stats_and_norm(b):
                """Load x(b), compute mean/rstd, write x_norm(b) in f32r."""
                xts = []
                partials = sb.tile([128, 2 * CT], F32, tag="part", bufs=2,
                                   name=f"part{b}")
                for t in range(CT):
                    xt = sb.tile([128, S], F32, tag="xload",
                                 bufs=cfg["xload_bufs"], name=f"x{b}_{t}")
                    nc.sync.dma_start(out=xt, in_=x_d[b, t * 128:(t + 1) * 128, :])
                    xts.append(xt)
                    sq = sb.tile([128, S], F32, tag="sqscr",
                                 bufs=cfg["sqscr_bufs"], name=f"sq{b}_{t}")
                    nc.scalar.activation(out=sq, in_=xt, func=ACT.Square,
                                         accum_out=partials[:, CT + t:CT + t + 1])
                    nc.vector.reduce_sum(out=partials[:, t:t + 1], in_=xt, axis=AX.X)
                partials_r = sb.tile([128, 2 * CT], F32R, tag="partr", bufs=2,
                                     name=f"partr{b}")
                nc.vector.tensor_copy(out=partials_r, in_=partials)
                stat_ps = ps.tile([1, 512], F32, tag="row", bufs=1, name=f"stat{b}")
                nc.tensor.matmul(stat_ps[0:1, 0:2 * CT], ones_t, partials_r,
                                 start=True, stop=True)
                scal = sb.tile([1, 5], F32, tag="scal", bufs=2, name=f"scal{b}")
                nc.vector.reduce_sum(out=scal[:, 3:4], in_=stat_ps[0:1, 0:CT], axis=AX.X)
                nc.vector.reduce_sum(out=scal[:, 4:5], in_=stat_ps[0:1, CT:2 * CT],
                                     axis=AX.X)
                nc.vector.tensor_scalar_mul(scal[:, 0:1], scal[:, 3:4], 1.0 / N_ELEM)
                nc.vector.tensor_scalar_mul(scal[:, 4:5], scal[:, 4:5], 1.0 / N_ELEM)
                nc.vector.tensor_tensor(out=scal[:, 3:4], in0=scal[:, 0:1],
                                        in1=scal[:, 0:1], op=OP.mult)
                nc.vector.tensor_tensor(out=scal[:, 2:3], in0=scal[:, 4:5],
                                        in1=scal[:, 3:4], op=OP.subtract)
                nc.vector.tensor_scalar_add(scal[:, 2:3], scal[:, 2:3], EPS)
                nc.vector.reciprocal(out=scal[:, 1:2], in_=scal[:, 2:3])
                for _ in range(3):
                    nc.vector.tensor_tensor(out=scal[:, 3:4], in0=scal[:, 1:2],
                                            in1=scal[:, 1:2], op=OP.mult)
                    nc.vector.tensor_tensor(out=scal[:, 3:4], in0=scal[:, 3:4],
                                            in1=scal[:, 2:3], op=OP.mult)
                    nc.vector.tensor_scalar(scal[:, 3:4], scal[:, 3:4], -0.5, 1.5,
                                            op0=OP.mult, op1=OP.add)
                    nc.vector.tensor_tensor(out=scal[:, 1:2], in0=scal[:, 1:2],
                                            in1=scal[:, 3:4], op=OP.mult)
                nc.sync.dma_start(out=scr_ms[b], in_=scal[0:1, 0:2])
                mbc = sb.tile([128, 2], F32, tag="mbc", bufs=2, name=f"mbc{b}")
                nc.sync.dma_start(
                    out=mbc,
                    in_=scr_ms[b].rearrange("(o s) -> o s", o=1).partition_broadcast(128))
                xns = []
                for t in range(CT):
                    xn = sb.tile([128, S], F32R, tag="xn", bufs=cfg["xn_bufs"],
                                 name=f"xn{b}_{t}")
                    nc.vector.tensor_scalar(xn, xts[t], mbc[:, 0:1], mbc[:, 1:2],
                                            op0=OP.subtract, op1=OP.mult)
                    xns.append(xn)
                return xns

            def qkv(b, xns):
                qk = {}
                for m in (0, 4, 1, 5, 2, 6, 3, 7):
                    qt = sb.tile([128, S], F32R, tag="qk", bufs=cfg["qk_bufs"],
                                 name=f"qk{b}_{m}")
                    for ch in range(NCH):
                        mm = ps.tile([128, 512], F32, tag="mm", bufs=cfg["mm_bufs"],
                                     name=f"mmq{b}_{m}_{ch}")
                        for k in range(CT):
                            nc.tensor.matmul(mm, wq[k][:, m * 128:(m + 1) * 128],
                                             xns[k][:, ch * 512:(ch + 1) * 512],
                                             start=(k == 0), stop=(k == CT - 1))
                        nc.vector.tensor_scalar_add(qt[:, ch * 512:(ch + 1) * 512],
                                                    mm, bqkv_t[:, m:m + 1])
                    qk[m] = qt
                vts = []
                for st in range(ST):
                    vt = sb.tile([128, C], F32R, tag="vt", bufs=cfg["vt_bufs"],
                                 name=f"vt{b}_{st}")
                    mm = ps.tile([128, 512], F32, tag="mm", bufs=cfg["mm_bufs"],
                                 name=f"mmv{b}_{st}")
                    for k in range(CT):
                        nc.tensor.matmul(mm, xns[k][:, st * 128:(st + 1) * 128],
                                         wq[k][:, 2 * C:3 * C],
                                         start=(k == 0), stop=(k == CT - 1))
                    if use_v_bias:
                        nc.vector.scalar_tensor_tensor(out=vt, in0=mm, scalar=0.0,
                                                       in1=bv_bc, op0=OP.add, op1=OP.add)
                    else:
                        nc.vector.tensor_copy(out=vt, in_=mm)
                    vts.append(vt)
                return qk, vts

            def attention(b, qk, vts):
                on = []
                for h in range(NH):
                    ot = sb.tile([128, S], F32R, tag="on", bufs=cfg["on_bufs"],
                                 name=f"on{b}_{h}")
                    on.append(ot)
                for h in range(NH):
                    q_t, k_t = qk[h], qk[NH + h]
                    ets = []
                    for st in range(ST):
                        sc = ps.tile([128, S], F32, tag="sc", bufs=1,
                                     name=f"sc{b}_{h}_{st}")
                        for ch in range(NCH):
                            nc.tensor.matmul(sc[:, ch * 512:(ch + 1) * 512],
                                             k_t[:, st * 128:(st + 1) * 128],
                                             q_t[:, ch * 512:(ch + 1) * 512],
                                             start=True, stop=True)
                        et = sb.tile([128, S], F32R, tag="et", bufs=cfg["et_bufs"],
                                     name=f"et{b}_{h}_{st}")
                        nc.scalar.activation(out=et, in_=sc, func=ACT.Exp, scale=SCALE)
                        ets.append(et)
                    for ch in range(NCH):
                        chs = slice(ch * 512, (ch + 1) * 512)
                        row = ps.tile([1, 512], F32, tag="row", bufs=1,
                                      name=f"row{b}_{h}_{ch}")
                        for st in range(ST):
                            nc.tensor.matmul(row, ones_t, ets[st][:, chs],
                                             start=(st == 0), stop=(st == ST - 1))
                        rcp = sb.tile([1, 512], F32, tag="rcp", bufs=2,
                                      name=f"rcp{b}_{h}_{ch}")
                        nc.vector.reciprocal(out=rcp, in_=row)
                        nc.sync.dma_start(out=scr_rcp[b, h, ch], in_=rcp)
                        rbc = sb.tile([128, 512], F32, tag="rbc", bufs=2,
                                      name=f"rbc{b}_{h}_{ch}")
                        nc.sync.dma_start(
                            out=rbc,
                            in_=scr_rcp[b, h, ch].rearrange("(o s) -> o s", o=1)
                            .partition_broadcast(128))
                        av = ps.tile([128, 512], F32, tag="av", bufs=cfg["av_bufs"],
                                     name=f"av{b}_{h}_{ch}")
                        for st in range(ST):
                            nc.tensor.matmul(av, vts[st][:, h * HD:(h + 1) * HD],
                                             ets[st][:, chs],
                                             start=(st == 0), stop=(st == ST - 1))
                        nc.vector.tensor_tensor(out=on[h][:, chs], in0=av, in1=rbc,
                                                op=OP.mult)
                return on

            def outproj(b, on):
                for m in range(CT):
                    rx = sb.tile([128, S], F32, tag="rx", bufs=cfg["rx_bufs"],
                                 name=f"rx{b}_{m}")
                    nc.sync.dma_start(out=rx, in_=x_d[b, m * 128:(m + 1) * 128, :])
                    res = sb.tile([128, S], F32, tag="res", bufs=cfg["res_bufs"],
                                  name=f"res{b}_{m}")
                    for ch in range(NCH):
                        mm = ps.tile([128, 512], F32, tag="mm", bufs=cfg["mm_bufs"],
                                     name=f"mmo{b}_{m}_{ch}")
                        for k in range(CT):
                            nc.tensor.matmul(mm, wo[k][:, m * 128:(m + 1) * 128],
                                             on[k][:, ch * 512:(ch + 1) * 512],
                                             start=(k == 0), stop=(k == CT - 1))
                        nc.vector.scalar_tensor_tensor(
                            out=res[:, ch * 512:(ch + 1) * 512], in0=mm,
                            scalar=bout_t[:, m:m + 1],
                            in1=rx[:, ch * 512:(ch + 1) * 512],
                            op0=OP.add, op1=OP.add)
                    nc.sync.dma_start(out=y_d[b, m * 128:(m + 1) * 128, :], in_=res)

            xns0 = stats_and_norm(0)
            qk0, vts0 = qkv(0, xns0)
            xns1 = stats_and_norm(1)   # ACT/DVE overlap QKV(0) on PE
            on0 = attention(0, qk0, vts0)
            outproj(0, on0)
            qk1, vts1 = qkv(1, xns1)   # fills PE gaps during attention(0)
            on1 = attention(1, qk1, vts1)
            outproj(1, on1)
    nc.finalize()
    return nc


def build_program_v3(use_v_bias: bool, cfg: dict | None = None) -> bass.Bass:
    """v1 shapes ([128,512] exp, sc bufs 2) with fine-grained interleaved
    emission: the Tile scheduler allocates pool slots in emission order, so
    batch-1 stats/QKV are emitted BETWEEN batch-0 attention heads to fill the
    PE gaps that ACT exp pacing leaves.
    """
    cfg = {**DEFAULT_CFG, "xn_bufs": 8, "xload_bufs": 4, **(cfg or {})}
    nc = bacc.Bacc()
    x_d = nc.dram_tensor("x", [BPC, C, S], F32, kind="ExternalInput")
    wqkv_d = nc.dram_tensor("wqkvT", [C, 3 * C], F32, kind="ExternalInput")
    bqkv_d = nc.dram_tensor("bqkv", [3 * C], F32, kind="ExternalInput")
    wout_d = nc.dram_tensor("woutT", [C, C], F32, kind="ExternalInput")
    bout_d = nc.dram_tensor("bout", [C], F32, kind="ExternalInput")
    y_d = nc.dram_tensor("y", [BPC, C, S], F32, kind="ExternalOutput")
    scr_ms = nc.dram_tensor("scr_ms", [BPC, 2], F32)
    scr_rcp = nc.dram_tensor("scr_rcp", [BPC, NH, NCH, 512], F32)

    with tile.TileContext(nc) as tc:
        with (
            tc.tile_pool(name="const", bufs=1) as cpool,
            tc.tile_pool(name="sb", bufs=1) as sb,
            tc.tile_pool(name="ps", bufs=1, space="PSUM") as ps,
        ):
            wq = []
            for k in range(CT):
                t = cpool.tile([128, 3 * C], F32R, name=f"wq{k}")
                nc.gpsimd.dma_start(out=t, in_=wqkv_d[k * 128:(k + 1) * 128, :])
                wq.append(t)
            wo = []
            for k in range(CT):
                t = cpool.tile([128, C], F32R, name=f"wo{k}")
                nc.gpsimd.dma_start(out=t, in_=wout_d[k * 128:(k + 1) * 128, :])
                wo.append(t)
            bqkv_t = cpool.tile([128, 12], F32, name="bqkv_t")
            nc.sync.dma_start(out=bqkv_t, in_=bqkv_d[:].rearrange("(m p) -> p m", p=128))
            bout_t = cpool.tile([128, CT], F32, name="bout_t")
            nc.sync.dma_start(out=bout_t, in_=bout_d[:].rearrange("(m p) -> p m", p=128))
            ones32 = cpool.tile([128, 1], F32, name="ones32")
            nc.vector.memset(ones32, 1.0)
            ones_t = cpool.tile([128, 1], F32R, name="ones_t")
            nc.vector.tensor_copy(out=ones_t, in_=ones32)
            if use_v_bias:
                bv_bc = cpool.tile([128, C], F32, name="bv_bc")
                nc.sync.dma_start(
                    out=bv_bc,
                    in_=bqkv_d[2 * C:3 * C].rearrange("(o s) -> o s", o=1)
                    .partition_broadcast(128))

            def stats_and_norm(b):
                xts = []
                partials = sb.tile([128, 2 * CT], F32, tag="part", bufs=2,
                                   name=f"part{b}")
                for t in range(CT):
                    xt = sb.tile([128, S], F32, tag="xload",
                                 bufs=cfg["xload_bufs"], name=f"x{b}_{t}")
                    nc.sync.dma_start(out=xt, in_=x_d[b, t * 128:(t + 1) * 128, :])
                    xts.append(xt)
                    sq = sb.tile([128, S], F32, tag="sqscr", bufs=1, name=f"sq{b}_{t}")
                    nc.scalar.activation(out=sq, in_=xt, func=ACT.Square,
                                         accum_out=partials[:, CT + t:CT + t + 1])
                    nc.vector.reduce_sum(out=partials[:, t:t + 1], in_=xt, axis=AX.X)
                partials_r = sb.tile([128, 2 * CT], F32R, tag="partr", bufs=2,
                                     name=f"partr{b}")
                nc.vector.tensor_copy(out=partials_r, in_=partials)
                stat_ps = ps.tile([1, 512], F32, tag="row", bufs=1, name=f"stat{b}")
                nc.tensor.matmul(stat_ps[0:1, 0:2 * CT], ones_t, partials_r,
                                 start=True, stop=True)
                scal = sb.tile([1, 5], F32, tag="scal", bufs=2, name=f"scal{b}")
                nc.vector.reduce_sum(out=scal[:, 3:4], in_=stat_ps[0:1, 0:CT], axis=AX.X)
                nc.vector.reduce_sum(out=scal[:, 4:5], in_=stat_ps[0:1, CT:2 * CT],
                                     axis=AX.X)
                nc.vector.tensor_scalar_mul(scal[:, 0:1], scal[:, 3:4], 1.0 / N_ELEM)
                nc.vector.tensor_scalar_mul(scal[:, 4:5], scal[:, 4:5], 1.0 / N_ELEM)
                nc.vector.tensor_tensor(out=scal[:, 3:4], in0=scal[:, 0:1],
                                        in1=scal[:, 0:1], op=OP.mult)
                nc.vector.tensor_tensor(out=scal[:, 2:3], in0=scal[:, 4:5],
                                        in1=scal[:, 3:4], op=OP.subtract)
                nc.vector.tensor_scalar_add(scal[:, 2:3], scal[:, 2:3], EPS)
                nc.vector.reciprocal(out=scal[:, 1:2], in_=scal[:, 2:3])
                for _ in range(3):
                    nc.vector.tensor_tensor(out=scal[:, 3:4], in0=scal[:, 1:2],
                                            in1=scal[:, 1:2], op=OP.mult)
                    nc.vector.tensor_tensor(out=scal[:, 3:4], in0=scal[:, 3:4],
                                            in1=scal[:, 2:3], op=OP.mult)
                    nc.vector.tensor_scalar(scal[:, 3:4], scal[:, 3:4], -0.5, 1.5,
                                            op0=OP.mult, op1=OP.add)
                    nc.vector.tensor_tensor(out=scal[:, 1:2], in0=scal[:, 1:2],
                                            in1=scal[:, 3:4], op=OP.mult)
                nc.sync.dma_start(out=scr_ms[b], in_=scal[0:1, 0:2])
                mbc = sb.tile([128, 2], F32, tag="mbc", bufs=2, name=f"mbc{b}")
                nc.sync.dma_start(
                    out=mbc,
                    in_=scr_ms[b].rearrange("(o s) -> o s", o=1).partition_broadcast(128))
                xns = []
                for t in range(CT):
                    xn = sb.tile([128, S], F32R, tag="xn", bufs=cfg["xn_bufs"],
                                 name=f"xn{b}_{t}")
                    nc.vector.tensor_scalar(xn, xts[t], mbc[:, 0:1], mbc[:, 1:2],
                                            op0=OP.subtract, op1=OP.mult)
                    xns.append(xn)
                return xns

            def qkv_mtile(b, m, xns):
                qt = sb.tile([128, S], F32R, tag="qk", bufs=cfg["qk_bufs"],
                             name=f"qk{b}_{m}")
                for ch in range(NCH):
                    mm = ps.tile([128, 512], F32, tag="mm", bufs=cfg["mm_bufs"],
                                 name=f"mmq{b}_{m}_{ch}")
                    for k in range(CT):
                        nc.tensor.matmul(mm, wq[k][:, m * 128:(m + 1) * 128],
                                         xns[k][:, ch * 512:(ch + 1) * 512],
                                         start=(k == 0), stop=(k == CT - 1))
                    nc.vector.tensor_scalar_add(qt[:, ch * 512:(ch + 1) * 512],
                                                mm, bqkv_t[:, m:m + 1])
                return qt

            def vt_stile(b, st, xns):
                vt = sb.tile([128, C], F32R, tag="vt", bufs=cfg["vt_bufs"],
                             name=f"vt{b}_{st}")
                mm = ps.tile([128, 512], F32, tag="mm", bufs=cfg["mm_bufs"],
                             name=f"mmv{b}_{st}")
                for k in range(CT):
                    nc.tensor.matmul(mm, xns[k][:, st * 128:(st + 1) * 128],
                                     wq[k][:, 2 * C:3 * C],
                                     start=(k == 0), stop=(k == CT - 1))
                if use_v_bias:
                    nc.vector.scalar_tensor_tensor(out=vt, in0=mm, scalar=0.0,
                                                   in1=bv_bc, op0=OP.add, op1=OP.add)
                else:
                    nc.vector.tensor_copy(out=vt, in_=mm)
                return vt

            def alloc_on(b):
                return [sb.tile([128, S], F32R, tag="on", bufs=cfg["on_bufs"],
                                name=f"on{b}_{h}") for h in range(NH)]

            def attention_head(b, h, q_t, k_t, vts, on):
                for ch in range(NCH):
                    ets = []
                    for st in range(ST):
                        sc = ps.tile([128, 512], F32, tag="sc", bufs=cfg["sc_bufs"],
                                     name=f"sc{b}_{h}_{ch}_{st}")
                        nc.tensor.matmul(sc, k_t[:, st * 128:(st + 1) * 128],
                                         q_t[:, ch * 512:(ch + 1) * 512],
                                         start=True, stop=True)
                        et = sb.tile([128, 512], F32R, tag="et", bufs=cfg["et_bufs"],
                                     name=f"et{b}_{h}_{ch}_{st}")
                        nc.scalar.activation(out=et, in_=sc, func=ACT.Exp, scale=SCALE)
                        ets.append(et)
                    row = ps.tile([1, 512], F32, tag="row", bufs=1,
                                  name=f"row{b}_{h}_{ch}")
                    for st in range(ST):
                        nc.tensor.matmul(row, ones_t, ets[st],
                                         start=(st == 0), stop=(st == ST - 1))
                    rcp = sb.tile([1, 512], F32, tag="rcp", bufs=2,
                                  name=f"rcp{b}_{h}_{ch}")
                    nc.vector.reciprocal(out=rcp, in_=row)
                    nc.sync.dma_start(out=scr_rcp[b, h, ch], in_=rcp)
                    rbc = sb.tile([128, 512], F32, tag="rbc", bufs=2,
                                  name=f"rbc{b}_{h}_{ch}")
                    nc.sync.dma_start(
                        out=rbc,
                        in_=scr_rcp[b, h, ch].rearrange("(o s) -> o s", o=1)
                        .partition_broadcast(128))
                    av = ps.tile([128, 512], F32, tag="av", bufs=cfg["av_bufs"],
                                 name=f"av{b}_{h}_{ch}")
                    for st in range(ST):
                        nc.tensor.matmul(av, vts[st][:, h * HD:(h + 1) * HD], ets[st],
                                         start=(st == 0), stop=(st == ST - 1))
                    nc.vector.tensor_tensor(out=on[h][:, ch * 512:(ch + 1) * 512],
                                            in0=av, in1=rbc, op=OP.mult)

            def outproj(b, on):
                for m in range(CT):
                    rx = sb.tile([128, S], F32, tag="rx", bufs=cfg["rx_bufs"],
                                 name=f"rx{b}_{m}")
                    nc.sync.dma_start(out=rx, in_=x_d[b, m * 128:(m + 1) * 128, :])
                    res = sb.tile([128, S], F32, tag="res", bufs=cfg["res_bufs"],
                                  name=f"res{b}_{m}")
                    for ch in range(NCH):
                        mm = ps.tile([128, 512], F32, tag="mm", bufs=cfg["mm_bufs"],
                                     name=f"mmo{b}_{m}_{ch}")
                        for k in range(CT):
                            nc.tensor.matmul(mm, wo[k][:, m * 128:(m + 1) * 128],
                                             on[k][:, ch * 512:(ch + 1) * 512],
                                             start=(k == 0), stop=(k == CT - 1))
                        nc.vector.scalar_tensor_tensor(
                            out=res[:, ch * 512:(ch + 1) * 512], in0=mm,
                            scalar=bout_t[:, m:m + 1],
                            in1=rx[:, ch * 512:(ch + 1) * 512],
                            op0=OP.add, op1=OP.add)
                    nc.sync.dma_start(out=y_d[b, m * 128:(m + 1) * 128, :], in_=res)

            # batch 0 front
            xns0 = stats_and_norm(0)
            qk0 = {}
            for m in (0, 4, 1, 5, 2, 6, 3, 7):
                qk0[m] = qkv_mtile(0, m, xns0)
            vts0 = [vt_stile(0, st, xns0) for st in range(ST)]
            on0 = alloc_on(0)
            # attention(0) with batch-1 prep interleaved between heads
            attention_head(0, 0, qk0[0], qk0[4], vts0, on0)
            xns1 = stats_and_norm(1)
            attention_head(0, 1, qk0[1], qk0[5], vts0, on0)
            qk1 = {}
            qk1[0] = qkv_mtile(1, 0, xns1)
            qk1[4] = qkv_mtile(1, 4, xns1)
            attention_head(0, 2, qk0[2], qk0[6], vts0, on0)
            for m in (1, 5, 2, 6):
                qk1[m] = qkv_mtile(1, m, xns1)
            attention_head(0, 3, qk0[3], qk0[7], vts0, on0)
            for m in (3, 7):
                qk1[m] = qkv_mtile(1, m, xns1)
            vts1 = [vt_stile(1, st, xns1) for st in range(ST)]
            outproj(0, on0)
            on1 = alloc_on(1)
            for h in range(NH):
                attention_head(1, h, qk1[h], qk1[NH + h], vts1, on1)
            outproj(1, on1)
    nc.finalize()
    return nc


_cached = {}


def _get_program(use_v_bias: bool) -> bass.Bass:
    if use_v_bias not in _cached:
        _cached[use_v_bias] = build_program(use_v_bias)
    return _cached[use_v_bias]


def kernel(x, gn_weight, gn_bias, qkv_w, qkv_b, out_w, out_b):
    x = np.ascontiguousarray(np.asarray(x, dtype=np.float32))
    gn_weight = np.asarray(gn_weight, dtype=np.float32)
    gn_bias = np.asarray(gn_bias, dtype=np.float32)
    qkv_w = np.asarray(qkv_w, dtype=np.float32)
    qkv_b = np.asarray(qkv_b, dtype=np.float32)
    out_w = np.asarray(out_w, dtype=np.float32)
    out_b = np.asarray(out_b, dtype=np.float32)

    # fold the GroupNorm affine into the QKV projection (host-side prep)
    w_eff = qkv_w * gn_weight[None, :]
    b_eff = qkv_b + qkv_w @ gn_bias
    wqkvT = np.ascontiguousarray(w_eff.T)            # [C, 3C]
    woutT = np.ascontiguousarray(out_w.T)            # [C, C]
    use_v_bias = bool(np.any(b_eff[2 * C:] != 0.0))

    nc = _get_program(use_v_bias)
    xs = x.reshape(B, C, S)
    in_maps = []
    for c in range(N_CORES):
        in_maps.append({
            "x": np.ascontiguousarray(xs[c * BPC:(c + 1) * BPC]),
            "wqkvT": wqkvT,
            "bqkv": np.ascontiguousarray(b_eff),
            "woutT": woutT,
            "bout": np.ascontiguousarray(out_b),
        })
    r = run_bass_kernel_spmd(nc, in_maps, list(range(N_CORES)))
    out = np.concatenate([r.results[c]["y"] for c in range(N_CORES)], axis=0)
    return out.reshape(B, C, H, W).astype(np.float32)


# revision 23
# speedup vs baseline: 291.3817x; 291.3817x over previous
"""AttentionBlock (GroupNorm -> MHA -> out-proj -> residual) on 8 TRN2 NeuronCores.

Problem: x (16, 512, 32, 32) fp32; GroupNorm(groups=1) over (C,H,W); spatial
flattened to a 1024-token sequence; 4 heads x 128 dim self-attention; output
projection; residual add.

Sharding: pure data-parallel over batch - 2 batch elements per core, no
collectives. Each core runs the identical program on its own x shard.

Layout strategy (per batch element, everything channel-major [c, s] so the PE
contracts over partitions with zero transposes):
  - GroupNorm stats via ACT Square+accum / DVE row-reduce, cross-partition via
    a ones-vector matmul; rstd by Newton rsqrt on DVE (avoids ACT table
    switch away from the Exp set). Partition broadcasts are 0-stride-AP DMAs.
  - QKV: qkv_cs[m, s] = qkv_wT.T @ x_norm  (Q, K stay [c, s]); V is computed
    directly transposed, vT[s, vd] = x_norm.T @ qkv_wT_v, so attention needs
    no transposes at all.
  - scoresT[s2, s1] = K.T @ Q per head; exp on ACT (PSUM -> SBUF, fused
    1/sqrt(hd) scale; softmax max-subtraction skipped - scores are O(1) by
    construction so exp cannot overflow).
  - row sums of exp via ones-vector matmul (PSUM accumulation over s2 tiles);
    normalization applied to the (small) AV output, with the reciprocal
    broadcast across partitions by GPSIMD.
  - out-proj + residual fused into the PSUM-evacuation op on DVE.
  - All matmuls run in float32r (rounded-fp32, full PE rate at N=512).

GroupNorm's affine (gn_weight/gn_bias) is folded into the QKV weights on the
host: qkv = W @ (xn * g + b) = (W * g) @ xn + (qkv_b + W @ b).
"""
import sys

sys.path.insert(0, "/opt/trn_rl_repo")

import numpy as np

import concourse.bass as bass
import concourse.mybir as mybir
import concourse.tile as tile
from concourse import bacc
from concourse.bass_utils import run_bass_kernel_spmd

F32 = mybir.dt.float32
F32R = mybir.dt.float32r
AX = mybir.AxisListType
OP = mybir.AluOpType
ACT = mybir.ActivationFunctionType

N_CORES = 8
B, C, H, W = 16, 512, 32, 32
S = H * W                     # 1024 sequence positions
NH, HD = 4, C // 4            # 4 heads x 128
BPC = B // N_CORES            # 2 batch elements per core
CT = C // 128                 # 4 channel tiles
ST = S // 128                 # 8 sequence tiles
NCH = S // 512                # 2 free-dim chunks of 512
EPS = 1e-5
SCALE = 1.0 / float(np.sqrt(HD))
N_ELEM = float(C * S)


DEFAULT_CFG = {
    "sc_bufs": 2, "av_bufs": 2, "mm_bufs": 3, "et_bufs": 12,
    "xn_bufs": 4, "qk_bufs": 8, "vt_bufs": 8, "on_bufs": 4,
    "xload_bufs": 5, "res_bufs": 2, "rx_bufs": 2,
    # debug/timing-shape flags (change semantics; model experiments only)
    "skip_norm": False, "exp_on_dve": False,
}


def build_program(use_v_bias: bool, cfg: dict | None = None) -> bass.Bass:
    cfg = {**DEFAULT_CFG, **(cfg or {})}
    nc = bacc.Bacc()
    x_d = nc.dram_tensor("x", [BPC, C, S], F32, kind="ExternalInput")
    wqkv_d = nc.dram_tensor("wqkvT", [C, 3 * C], F32, kind="ExternalInput")
    bqkv_d = nc.dram_tensor("bqkv", [3 * C], F32, kind="ExternalInput")
    wout_d = nc.dram_tensor("woutT", [C, C], F32, kind="ExternalInput")
    bout_d = nc.dram_tensor("bout", [C], F32, kind="ExternalInput")
    y_d = nc.dram_tensor("y", [BPC, C, S], F32, kind="ExternalOutput")
    # DRAM scratch for partition broadcasts (SBUF->DRAM->0-stride-read-back)
    scr_ms = nc.dram_tensor("scr_ms", [BPC, 2], F32)
    scr_rcp = nc.dram_tensor("scr_rcp", [BPC, NH, NCH, 512], F32)

    with tile.TileContext(nc) as tc:
        with (
            tc.tile_pool(name="const", bufs=1) as cpool,
            tc.tile_pool(name="sb", bufs=1) as sb,
            tc.tile_pool(name="ps", bufs=1, space="PSUM") as ps,
        ):
            # ---- constants ----
            wq = []
            for k in range(CT):
                t = cpool.tile([128, 3 * C], F32R, name=f"wq{k}")
                nc.gpsimd.dma_start(out=t, in_=wqkv_d[k * 128:(k + 1) * 128, :])
                wq.append(t)
            wo = []
            for k in range(CT):
                t = cpool.tile([128, C], F32R, name=f"wo{k}")
                nc.gpsimd.dma_start(out=t, in_=wout_d[k * 128:(k + 1) * 128, :])
                wo.append(t)
            bqkv_t = cpool.tile([128, 12], F32, name="bqkv_t")
            nc.sync.dma_start(out=bqkv_t, in_=bqkv_d[:].rearrange("(m p) -> p m", p=128))
            bout_t = cpool.tile([128, CT], F32, name="bout_t")
            nc.sync.dma_start(out=bout_t, in_=bout_d[:].rearrange("(m p) -> p m", p=128))
            ones32 = cpool.tile([128, 1], F32, name="ones32")
            nc.vector.memset(ones32, 1.0)
            ones_t = cpool.tile([128, 1], F32R, name="ones_t")
            nc.vector.tensor_copy(out=ones_t, in_=ones32)
            if use_v_bias:
                bv_bc = cpool.tile([128, C], F32, name="bv_bc")
                nc.sync.dma_start(
                    out=bv_bc,
                    in_=bqkv_d[2 * C:3 * C].rearrange("(o s) -> o s", o=1)
                    .partition_broadcast(128))

            for b in range(BPC):
                # ---- GroupNorm statistics ----
                xts = []
                partials = sb.tile([128, 2 * CT], F32, tag="part", bufs=2, name=f"part{b}")
                for t in range(CT):
                    xt = sb.tile([128, S], F32, tag="xload", bufs=cfg["xload_bufs"], name=f"x{b}_{t}")
                    nc.sync.dma_start(out=xt, in_=x_d[b, t * 128:(t + 1) * 128, :])
                    xts.append(xt)
                    sq = sb.tile([128, S], F32, tag="sqscr", bufs=2, name=f"sq{b}_{t}")
                    nc.scalar.activation(out=sq, in_=xt, func=ACT.Square,
                                         accum_out=partials[:, CT + t:CT + t + 1])
                    nc.vector.reduce_sum(out=partials[:, t:t + 1], in_=xt, axis=AX.X)
                partials_r = sb.tile([128, 2 * CT], F32R, tag="partr", bufs=2,
                                     name=f"partr{b}")
                nc.vector.tensor_copy(out=partials_r, in_=partials)
                stat_ps = ps.tile([1, 512], F32, tag="row", bufs=1, name=f"stat{b}")
                nc.tensor.matmul(stat_ps[0:1, 0:2 * CT], ones_t, partials_r,
                                 start=True, stop=True)
                # scalar chain on partition 0; cols: 0=mean 1=y(rstd) 2=v 3,4=tmp
                scal = sb.tile([1, 5], F32, tag="scal", bufs=2, name=f"scal{b}")
                nc.vector.reduce_sum(out=scal[:, 3:4], in_=stat_ps[0:1, 0:CT], axis=AX.X)
                nc.vector.reduce_sum(out=scal[:, 4:5], in_=stat_ps[0:1, CT:2 * CT], axis=AX.X)
                nc.vector.tensor_scalar_mul(scal[:, 0:1], scal[:, 3:4], 1.0 / N_ELEM)
                nc.vector.tensor_scalar_mul(scal[:, 4:5], scal[:, 4:5], 1.0 / N_ELEM)
                nc.vector.tensor_tensor(out=scal[:, 3:4], in0=scal[:, 0:1],
                                        in1=scal[:, 0:1], op=OP.mult)
                nc.vector.tensor_tensor(out=scal[:, 2:3], in0=scal[:, 4:5],
                                        in1=scal[:, 3:4], op=OP.subtract)
                nc.vector.tensor_scalar_add(scal[:, 2:3], scal[:, 2:3], EPS)
                # Newton rsqrt: y0 = 1/v, y <- y*(1.5 - 0.5*v*y^2), 3 iters
                nc.vector.reciprocal(out=scal[:, 1:2], in_=scal[:, 2:3])
                for _ in range(3):
                    nc.vector.tensor_tensor(out=scal[:, 3:4], in0=scal[:, 1:2],
                                            in1=scal[:, 1:2], op=OP.mult)
                    nc.vector.tensor_tensor(out=scal[:, 3:4], in0=scal[:, 3:4],
                                            in1=scal[:, 2:3], op=OP.mult)
                    nc.vector.tensor_scalar(scal[:, 3:4], scal[:, 3:4], -0.5, 1.5,
                                            op0=OP.mult, op1=OP.add)
                    nc.vector.tensor_tensor(out=scal[:, 1:2], in0=scal[:, 1:2],
                                            in1=scal[:, 3:4], op=OP.mult)
                # broadcast (mean, rstd) to all partitions via a DRAM bounce
                nc.sync.dma_start(out=scr_ms[b], in_=scal[0:1, 0:2])
                mbc = sb.tile([128, 2], F32, tag="mbc", bufs=2, name=f"mbc{b}")
                nc.sync.dma_start(
                    out=mbc,
                    in_=scr_ms[b].rearrange("(o s) -> o s", o=1).partition_broadcast(128))

                # ---- x_norm = (x - mean) * rstd, written as float32r ----
                xns = []
                for t in range(CT):
                    xn = sb.tile([128, S], F32R, tag="xn", bufs=cfg["xn_bufs"], name=f"xn{b}_{t}")
                    nc.vector.tensor_scalar(xn, xts[t], mbc[:, 0:1], mbc[:, 1:2],
                                            op0=OP.subtract, op1=OP.mult)
                    xns.append(xn)

                # ---- QKV projections: Q,K channel-major; V sequence-major ----
                qk = {}
                for m in (0, 4, 1, 5, 2, 6, 3, 7):
                    qt = sb.tile([128, S], F32R, tag="qk", bufs=cfg["qk_bufs"], name=f"qk{b}_{m}")
                    for ch in range(NCH):
                        mm = ps.tile([128, 512], F32, tag="mm", bufs=cfg["mm_bufs"], name=f"mmq{b}_{m}_{ch}")
                        for k in range(CT):
                            nc.tensor.matmul(mm, wq[k][:, m * 128:(m + 1) * 128],
                                             xns[k][:, ch * 512:(ch + 1) * 512],
                                             start=(k == 0), stop=(k == CT - 1))
                        nc.vector.tensor_scalar_add(qt[:, ch * 512:(ch + 1) * 512], mm,
                                                    bqkv_t[:, m:m + 1])
                    qk[m] = qt
                vts = []
                for st in range(ST):
                    vt = sb.tile([128, C], F32R, tag="vt", bufs=cfg["vt_bufs"], name=f"vt{b}_{st}")
                    mm = ps.tile([128, 512], F32, tag="mm", bufs=cfg["mm_bufs"], name=f"mmv{b}_{st}")
                    for k in range(CT):
                        nc.tensor.matmul(mm, xns[k][:, st * 128:(st + 1) * 128],
                                         wq[k][:, 2 * C:3 * C],
                                         start=(k == 0), stop=(k == CT - 1))
                    if use_v_bias:
                        nc.vector.scalar_tensor_tensor(out=vt, in0=mm, scalar=0.0,
                                                       in1=bv_bc, op0=OP.add, op1=OP.add)
                    else:
                        nc.vector.tensor_copy(out=vt, in_=mm)
                    vts.append(vt)

                # ---- attention, head by head ----
                on = []
                for h in range(NH):
                    ot = sb.tile([128, S], F32R, tag="on", bufs=cfg["on_bufs"], name=f"on{b}_{h}")
                    on.append(ot)
                for h in range(NH):
                    q_t, k_t = qk[h], qk[NH + h]
                    for ch in range(NCH):
                        ets = []
                        for st in range(ST):
                            sc = ps.tile([128, 512], F32, tag="sc", bufs=cfg["sc_bufs"],
                                         name=f"sc{b}_{h}_{ch}_{st}")
                            nc.tensor.matmul(sc, k_t[:, st * 128:(st + 1) * 128],
                                             q_t[:, ch * 512:(ch + 1) * 512],
                                             start=True, stop=True)
                            et = sb.tile([128, 512], F32R, tag="et", bufs=cfg["et_bufs"],
                                         name=f"et{b}_{h}_{ch}_{st}")
                            if cfg["exp_on_dve"]:
                                nc.vector.tensor_copy(out=et, in_=sc)
                            else:
                                nc.scalar.activation(out=et, in_=sc, func=ACT.Exp, scale=SCALE)
                            ets.append(et)
                        if not cfg["skip_norm"]:
                            row = ps.tile([1, 512], F32, tag="row", bufs=1,
                                          name=f"row{b}_{h}_{ch}")
                            for st in range(ST):
                                nc.tensor.matmul(row, ones_t, ets[st],
                                                 start=(st == 0), stop=(st == ST - 1))
                            rcp = sb.tile([1, 512], F32, tag="rcp", bufs=2,
                                          name=f"rcp{b}_{h}_{ch}")
                            nc.vector.reciprocal(out=rcp, in_=row)
                            nc.sync.dma_start(out=scr_rcp[b, h, ch], in_=rcp)
                            rbc = sb.tile([128, 512], F32, tag="rbc", bufs=2,
                                          name=f"rbc{b}_{h}_{ch}")
                            nc.sync.dma_start(
                                out=rbc,
                                in_=scr_rcp[b, h, ch].rearrange("(o s) -> o s", o=1)
                                .partition_broadcast(128))
                        av = ps.tile([128, 512], F32, tag="av", bufs=cfg["av_bufs"],
                                     name=f"av{b}_{h}_{ch}")
                        for st in range(ST):
                            nc.tensor.matmul(av, vts[st][:, h * HD:(h + 1) * HD], ets[st],
                                             start=(st == 0), stop=(st == ST - 1))
                        if cfg["skip_norm"]:
                            nc.vector.tensor_copy(
                                out=on[h][:, ch * 512:(ch + 1) * 512], in_=av)
                        else:
                            nc.vector.tensor_tensor(out=on[h][:, ch * 512:(ch + 1) * 512],
                                                    in0=av, in1=rbc, op=OP.mult)

                # ---- output projection + residual ----
                for m in range(CT):
                    rx = sb.tile([128, S], F32, tag="rx", bufs=cfg["rx_bufs"], name=f"rx{b}_{m}")
                    nc.sync.dma_start(out=rx, in_=x_d[b, m * 128:(m + 1) * 128, :])
                    res = sb.tile([128, S], F32, tag="res", bufs=cfg["res_bufs"], name=f"res{b}_{m}")
                    for ch in range(NCH):
                        mm = ps.tile([128, 512], F32, tag="mm", bufs=cfg["mm_bufs"],
                                     name=f"mmo{b}_{m}_{ch}")
                        for k in range(CT):
                            nc.tensor.matmul(mm, wo[k][:, m * 128:(m + 1) * 128],
                                             on[k][:, ch * 512:(ch + 1) * 512],
                                             start=(k == 0), stop=(k == CT - 1))
                        nc.vector.scalar_tensor_tensor(
                            out=res[:, ch * 512:(ch + 1) * 512], in0=mm,
                            scalar=bout_t[:, m:m + 1],
                            in1=rx[:, ch * 512:(ch + 1) * 512],
                            op0=OP.add, op1=OP.add)
                    nc.sync.dma_start(out=y_d[b, m * 128:(m + 1) * 128, :], in_=res)
    nc.finalize()
    return nc


def build_program_v2(use_v_bias: bool, cfg: dict | None = None) -> bass.Bass:
    """Phased emission: stats(b1) overlaps QKV(b0) (ACT is idle there), QKV(b1)
    fills PE gaps of attention(b0), and exp runs on [128, 1024] PSUM reads
    (halves ACT per-instr overhead). PSUM banks: sc 1x2 + av 2 + mm 3 + row 1 = 8.
    """
    cfg = {**DEFAULT_CFG, "xn_bufs": 8, "et_bufs": 8, "res_bufs": 1,
           "sqscr_bufs": 1, "xload_bufs": 4, "rx_bufs": 1, **(cfg or {})}
    nc = bacc.Bacc()
    x_d = nc.dram_tensor("x", [BPC, C, S], F32, kind="ExternalInput")
    wqkv_d = nc.dram_tensor("wqkvT", [C, 3 * C], F32, kind="ExternalInput")
    bqkv_d = nc.dram_tensor("bqkv", [3 * C], F32, kind="ExternalInput")
    wout_d = nc.dram_tensor("woutT", [C, C], F32, kind="ExternalInput")
    bout_d = nc.dram_tensor("bout", [C], F32, kind="ExternalInput")
    y_d = nc.dram_tensor("y", [BPC, C, S], F32, kind="ExternalOutput")
    scr_ms = nc.dram_tensor("scr_ms", [BPC, 2], F32)
    scr_rcp = nc.dram_tensor("scr_rcp", [BPC, NH, NCH, 512], F32)

    with tile.TileContext(nc) as tc:
        with (
            tc.tile_pool(name="const", bufs=1) as cpool,
            tc.tile_pool(name="sb", bufs=1) as sb,
            tc.tile_pool(name="ps", bufs=1, space="PSUM") as ps,
        ):
            wq = []
            for k in range(CT):
                t = cpool.tile([128, 3 * C], F32R, name=f"wq{k}")
                nc.gpsimd.dma_start(out=t, in_=wqkv_d[k * 128:(k + 1) * 128, :])
                wq.append(t)
            wo = []
            for k in range(CT):
                t = cpool.tile([128, C], F32R, name=f"wo{k}")
                nc.gpsimd.dma_start(out=t, in_=wout_d[k * 128:(k + 1) * 128, :])
                wo.append(t)
            bqkv_t = cpool.tile([128, 12], F32, name="bqkv_t")
            nc.sync.dma_start(out=bqkv_t, in_=bqkv_d[:].rearrange("(m p) -> p m", p=128))
            bout_t = cpool.tile([128, CT], F32, name="bout_t")
            nc.sync.dma_start(out=bout_t, in_=bout_d[:].rearrange("(m p) -> p m", p=128))
            ones32 = cpool.tile([128, 1], F32, name="ones32")
            nc.vector.memset(ones32, 1.0)
            ones_t = cpool.tile([128, 1], F32R, name="ones_t")
            nc.vector.tensor_copy(out=ones_t, in_=ones32)
            if use_v_bias:
                bv_bc = cpool.tile([128, C], F32, name="bv_bc")
                nc.sync.dma_start(
                    out=bv_bc,
                    in_=bqkv_d[2 * C:3 * C].rearrange("(o s) -> o s", o=1)
                    .partition_broadcast(128))

            def stats_and_norm(b):
                """Load x(b), compute mean/rstd, write x_norm(b) in f32r."""
                xts = []
                partials = sb.tile([128, 2 * CT], F32, tag="part", bufs=2,
                                   name=f"part{b}")
                for t in range(CT):
                    xt = sb.tile([128, S], F32, tag="xload",
                                 bufs=cfg["xload_bufs"], name=f"x{b}_{t}")
                    nc.sync.dma_start(out=xt, in_=x_d[b, t * 128:(t + 1) * 128, :])
                    xts.append(xt)
                    sq = sb.tile([128, S], F32, tag="sqscr",
                                 bufs=cfg["sqscr_bufs"], name=f"sq{b}_{t}")
                    nc.scalar.activation(out=sq, in_=xt, func=ACT.Square,
                                         accum_out=partials[:, CT + t:CT + t + 1])
                    nc.vector.reduce_sum(out=partials[:, t:t + 1], in_=xt, axis=AX.X)
                partials_r = sb.tile([128, 2 * CT], F32R, tag="partr", bufs=2,
                                     name=f"partr{b}")
                nc.vector.tensor_copy(out=partials_r, in_=partials)
                stat_ps = ps.tile([1, 512], F32, tag="row", bufs=1, name=f"stat{b}")
                nc.tensor.matmul(stat_ps[0:1, 0:2 * CT], ones_t, partials_r,
                                 start=True, stop=True)
                scal = sb.tile([1, 5], F32, tag="scal", bufs=2, name=f"scal{b}")
                nc.vector.reduce_sum(out=scal[:, 3:4], in_=stat_ps[0:1, 0:CT], axis=AX.X)
                nc.vector.reduce_sum(out=scal[:, 4:5], in_=stat_ps[0:1, CT:2 * CT],
                                     axis=AX.X)
                nc.vector.tensor_scalar_mul(scal[:, 0:1], scal[:, 3:4], 1.0 / N_ELEM)
                nc.vector.tensor_scalar_mul(scal[:, 4:5], scal[:, 4:5], 1.0 / N_ELEM)
                nc.vector.tensor_tensor(out=scal[:, 3:4], in0=scal[:, 0:1],
                                        in1=scal[:, 0:1], op=OP.mult)
                nc.vector.tensor_tensor(out=scal[:, 2:3], in0=scal[:, 4:5],
                                        in1=scal[:, 3:4], op=OP.subtract)
                nc.vector.tensor_scalar_add(scal[:, 2:3], scal[:, 2:3], EPS)
                nc.vector.reciprocal(out=scal[:, 1:2], in_=scal[:, 2:3])
                for _ in range(3):
                    nc.vector.tensor_tensor(out=scal[:, 3:4], in0=scal[:, 1:2],
                                            in1=scal[:, 1:2], op=OP.mult)
                    nc.vector.tensor_tensor(out=scal[:, 3:4], in0=scal[:, 3:4],
                                            in1=scal[:, 2:3], op=OP.mult)
                    nc.vector.tensor_scalar(scal[:, 3:4], scal[:, 3:4], -0.5, 1.5,
                                            op0=OP.mult, op1=OP.add)
                    nc.vector.tensor_tensor(out=scal[:, 1:2], in0=scal[:, 1:2],
                                            in1=scal[:, 3:4], op=OP.mult)
                nc.sync.dma_start(out=scr_ms[b], in_=scal[0:1, 0:2])
                mbc = sb.tile([128, 2], F32, tag="mbc", bufs=2, name=f"mbc{b}")
                nc.sync.dma_start(
                    out=mbc,
                    in_=scr_ms[b].rearrange("(o s) -> o s", o=1).partition_broadcast(128))
                xns = []
                for t in range(CT):
                    xn = sb.tile([128, S], F32R, tag="xn", bufs=cfg["xn_bufs"],
                                 name=f"xn{b}_{t}")
                    nc.vector.tensor_scalar(xn, xts[t], mbc[:, 0:1], mbc[:, 1:2],
                                            op0=OP.subtract, op1=OP.mult)
                    xns.append(xn)
                return xns

            def qkv(b, xns):
                qk = {}
                for m in (0, 4, 1, 5, 2, 6, 3, 7):
                    qt = sb.tile([128, S], F32R, tag="qk", bufs=cfg["qk_bufs"],
                                 name=f"qk{b}_{m}")
                    for ch in range(NCH):
                        mm = ps.tile([128, 512], F32, tag="mm", bufs=cfg["mm_bufs"],
                                     name=f"mmq{b}_{m}_{ch}")
                        for k in range(CT):
                            nc.tensor.matmul(mm, wq[k][:, m * 128:(m + 1) * 128],
                                             xns[k][:, ch * 512:(ch + 1) * 512],
                                             start=(k == 0), stop=(k == CT - 1))
                        nc.vector.tensor_scalar_add(qt[:, ch * 512:(ch + 1) * 512],
                                                    mm, bqkv_t[:, m:m + 1])
                    qk[m] = qt
                vts = []
                for st in range(ST):
                    vt = sb.tile([128, C], F32R, tag="vt", bufs=cfg["vt_bufs"],
                                 name=f"vt{b}_{st}")
                    mm = ps.tile([128, 512], F32, tag="mm", bufs=cfg["mm_bufs"],
                                 name=f"mmv{b}_{st}")
                    for k in range(CT):
                        nc.tensor.matmul(mm, xns[k][:, st * 128:(st + 1) * 128],
                                         wq[k][:, 2 * C:3 * C],
                                         start=(k == 0), stop=(k == CT - 1))
                    if use_v_bias:
                        nc.vector.scalar_tensor_tensor(out=vt, in0=mm, scalar=0.0,
                                                       in1=bv_bc, op0=OP.add, op1=OP.add)
                    else:
                        nc.vector.tensor_copy(out=vt, in_=mm)
                    vts.append(vt)
                return qk, vts

            def attention(b, qk, vts):
                on = []
                for h in range(NH):
                    ot = sb.tile([128, S], F32R, tag="on", bufs=cfg["on_bufs"],
                                 name=f"on{b}_{h}")
                    on.append(ot)
                for h in range(NH):
                    q_t, k_t = qk[h], qk[NH + h]
                    ets = []
                    for st in range(ST):
                        sc = ps.tile([128, S], F32, tag="sc", bufs=1,
                                     name=f"sc{b}_{h}_{st}")
                        for ch in range(NCH):
                            nc.tensor.matmul(sc[:, ch * 512:(ch + 1) * 512],
                                             k_t[:, st * 128:(st + 1) * 128],
                                             q_t[:, ch * 512:(ch + 1) * 512],
                                             start=True, stop=True)
                        et = sb.tile([128, S], F32R, tag="et", bufs=cfg["et_bufs"],
                                     name=f"et{b}_{h}_{st}")
                        nc.scalar.activation(out=et, in_=sc, func=ACT.Exp, scale=SCALE)
                        ets.append(et)
                    for ch in range(NCH):
                        chs = slice(ch * 512, (ch + 1) * 512)
                        row = ps.tile([1, 512], F32, tag="row", bufs=1,
                                      name=f"row{b}_{h}_{ch}")
                        for st in range(ST):
                            nc.tensor.matmul(row, ones_t, ets[st][:, chs],
                                             start=(st == 0), stop=(st == ST - 1))
                        rcp = sb.tile([1, 512], F32, tag="rcp", bufs=2,
                                      name=f"rcp{b}_{h}_{ch}")
                        nc.vector.reciprocal(out=rcp, in_=row)
                        nc.sync.dma_start(out=scr_rcp[b, h, ch], in_=rcp)
                        rbc = sb.tile([128, 512], F32, tag="rbc", bufs=2,
                                      name=f"rbc{b}_{h}_{ch}")
                        nc.sync.dma_start(
                            out=rbc,
                            in_=scr_rcp[b, h, ch].rearrange("(o s) -> o s", o=1)
                            .partition_broadcast(128))
                        av = ps.tile([128, 512], F32, tag="av", bufs=cfg["av_bufs"],
                                     name=f"av{b}_{h}_{ch}")
                        for st in range(ST):
                            nc.tensor.matmul(av, vts[st][:, h * HD:(h + 1) * HD],
                                             ets[st][:, chs],
                                             start=(st == 0), stop=(st == ST - 1))
                        nc.vector.tensor_tensor(out=on[h][:, chs], in0=av, in1=rbc,
                                                op=OP.mult)
                return on

            def outproj(b, on):
                for m in range(CT):
                    rx = sb.tile([128, S], F32, tag="rx", bufs=cfg["rx_bufs"],
                                 name=f"rx{b}_{m}")
                    nc.sync.dma_start(out=rx, in_=x_d[b, m * 128:(m + 1) * 128, :])
                    res = sb.tile([128, S], F32, tag="res", bufs=cfg["res_bufs"],
                                  name=f"res{b}_{m}")
                    for ch in range(NCH):
                        mm = ps.tile([128, 512], F32, tag="mm", bufs=cfg["mm_bufs"],
                                     name=f"mmo{b}_{m}_{ch}")
                        for k in range(CT):
                            nc.tensor.matmul(mm, wo[k][:, m * 128:(m + 1) * 128],
                                             on[k][:, ch * 512:(ch + 1) * 512],
                                             start=(k == 0), stop=(k == CT - 1))
                        nc.vector.scalar_tensor_tensor(
                            out=res[:, ch * 512:(ch + 1) * 512], in0=mm,
                            scalar=bout_t[:, m:m + 1],
                            in1=rx[:, ch * 512:(ch + 1) * 512],
                            op0=OP.add, op1=OP.add)
                    nc.sync.dma_start(out=y_d[b, m * 128:(m + 1) * 128, :], in_=res)

            xns0 = stats_and_norm(0)
            qk0, vts0 = qkv(0, xns0)
            xns1 = stats_and_norm(1)   # ACT/DVE overlap QKV(0) on PE
            on0 = attention(0, qk0, vts0)
            outproj(0, on0)
            qk1, vts1 = qkv(1, xns1)   # fills PE gaps during attention(0)
            on1 = attention(1, qk1, vts1)
            outproj(1, on1)
    nc.finalize()
    return nc


def build_program_v3(use_v_bias: bool, cfg: dict | None = None) -> bass.Bass:
    """v1 shapes ([128,512] exp, sc bufs 2) with fine-grained interleaved
    emission: the Tile scheduler allocates pool slots in emission order, so
    batch-1 stats/QKV are emitted BETWEEN batch-0 attention heads to fill the
    PE gaps that ACT exp pacing leaves.
    """
    cfg = {**DEFAULT_CFG, "xn_bufs": 8, "xload_bufs": 4, **(cfg or {})}
    nc = bacc.Bacc()
    x_d = nc.dram_tensor("x", [BPC, C, S], F32, kind="ExternalInput")
    wqkv_d = nc.dram_tensor("wqkvT", [C, 3 * C], F32, kind="ExternalInput")
    bqkv_d = nc.dram_tensor("bqkv", [3 * C], F32, kind="ExternalInput")
    wout_d = nc.dram_tensor("woutT", [C, C], F32, kind="ExternalInput")
    bout_d = nc.dram_tensor("bout", [C], F32, kind="ExternalInput")
    y_d = nc.dram_tensor("y", [BPC, C, S], F32, kind="ExternalOutput")
    scr_ms = nc.dram_tensor("scr_ms", [BPC, 2], F32)
    scr_rcp = nc.dram_tensor("scr_rcp", [BPC, NH, NCH, 512], F32)

    with tile.TileContext(nc) as tc:
        with (
            tc.tile_pool(name="const", bufs=1) as cpool,
            tc.tile_pool(name="sb", bufs=1) as sb,
            tc.tile_pool(name="ps", bufs=1, space="PSUM") as ps,
        ):
            wq = []
            for k in range(CT):
                t = cpool.tile([128, 3 * C], F32R, name=f"wq{k}")
                nc.gpsimd.dma_start(out=t, in_=wqkv_d[k * 128:(k + 1) * 128, :])
                wq.append(t)
            wo = []
            for k in range(CT):
                t = cpool.tile([128, C], F32R, name=f"wo{k}")
                nc.gpsimd.dma_start(out=t, in_=wout_d[k * 128:(k + 1) * 128, :])
                wo.append(t)
            bqkv_t = cpool.tile([128, 12], F32, name="bqkv_t")
            nc.sync.dma_start(out=bqkv_t, in_=bqkv_d[:].rearrange("(m p) -> p m", p=128))
            bout_t = cpool.tile([128, CT], F32, name="bout_t")
            nc.sync.dma_start(out=bout_t, in_=bout_d[:].rearrange("(m p) -> p m", p=128))
            ones32 = cpool.tile([128, 1], F32, name="ones32")
            nc.vector.memset(ones32, 1.0)
            ones_t = cpool.tile([128, 1], F32R, name="ones_t")
            nc.vector.tensor_copy(out=ones_t, in_=ones32)
            if use_v_bias:
                bv_bc = cpool.tile([128, C], F32, name="bv_bc")
                nc.sync.dma_start(
                    out=bv_bc,
                    in_=bqkv_d[2 * C:3 * C].rearrange("(o s) -> o s", o=1)
                    .partition_broadcast(128))

            def stats_and_norm(b):
                xts = []
                partials = sb.tile([128, 2 * CT], F32, tag="part", bufs=2,
                                   name=f"part{b}")
                for t in range(CT):
                    xt = sb.tile([128, S], F32, tag="xload",
                                 bufs=cfg["xload_bufs"], name=f"x{b}_{t}")
                    nc.sync.dma_start(out=xt, in_=x_d[b, t * 128:(t + 1) * 128, :])
                    xts.append(xt)
                    sq = sb.tile([128, S], F32, tag="sqscr", bufs=1, name=f"sq{b}_{t}")
                    nc.scalar.activation(out=sq, in_=xt, func=ACT.Square,
                                         accum_out=partials[:, CT + t:CT + t + 1])
                    nc.vector.reduce_sum(out=partials[:, t:t + 1], in_=xt, axis=AX.X)
                partials_r = sb.tile([128, 2 * CT], F32R, tag="partr", bufs=2,
                                     name=f"partr{b}")
                nc.vector.tensor_copy(out=partials_r, in_=partials)
                stat_ps = ps.tile([1, 512], F32, tag="row", bufs=1, name=f"stat{b}")
                nc.tensor.matmul(stat_ps[0:1, 0:2 * CT], ones_t, partials_r,
                                 start=True, stop=True)
                scal = sb.tile([1, 5], F32, tag="scal", bufs=2, name=f"scal{b}")
                nc.vector.reduce_sum(out=scal[:, 3:4], in_=stat_ps[0:1, 0:CT], axis=AX.X)
                nc.vector.reduce_sum(out=scal[:, 4:5], in_=stat_ps[0:1, CT:2 * CT],
                                     axis=AX.X)
                nc.vector.tensor_scalar_mul(scal[:, 0:1], scal[:, 3:4], 1.0 / N_ELEM)
                nc.vector.tensor_scalar_mul(scal[:, 4:5], scal[:, 4:5], 1.0 / N_ELEM)
                nc.vector.tensor_tensor(out=scal[:, 3:4], in0=scal[:, 0:1],
                                        in1=scal[:, 0:1], op=OP.mult)
                nc.vector.tensor_tensor(out=scal[:, 2:3], in0=scal[:, 4:5],
                                        in1=scal[:, 3:4], op=OP.subtract)
                nc.vector.tensor_scalar_add(scal[:, 2:3], scal[:, 2:3], EPS)
                nc.vector.reciprocal(out=scal[:, 1:2], in_=scal[:, 2:3])
                for _ in range(3):
                    nc.vector.tensor_tensor(out=scal[:, 3:4], in0=scal[:, 1:2],
                                            in1=scal[:, 1:2], op=OP.mult)
                    nc.vector.tensor_tensor(out=scal[:, 3:4], in0=scal[:, 3:4],
                                            in1=scal[:, 2:3], op=OP.mult)
                    nc.vector.tensor_scalar(scal[:, 3:4], scal[:, 3:4], -0.5, 1.5,
                                            op0=OP.mult, op1=OP.add)
                    nc.vector.tensor_tensor(out=scal[:, 1:2], in0=scal[:, 1:2],
                                            in1=scal[:, 3:4], op=OP.mult)
                nc.sync.dma_start(out=scr_ms[b], in_=scal[0:1, 0:2])
                mbc = sb.tile([128, 2], F32, tag="mbc", bufs=2, name=f"mbc{b}")
                nc.sync.dma_start(
                    out=mbc,
                    in_=scr_ms[b].rearrange("(o s) -> o s", o=1).partition_broadcast(128))
                xns = []
                for t in range(CT):
                    xn = sb.tile([128, S], F32R, tag="xn", bufs=cfg["xn_bufs"],
                                 name=f"xn{b}_{t}")
                    nc.vector.tensor_scalar(xn, xts[t], mbc[:, 0:1], mbc[:, 1:2],
                                            op0=OP.subtract, op1=OP.mult)
                    xns.append(xn)
                return xns

            def qkv_mtile(b, m, xns):
                qt = sb.tile([128, S], F32R, tag="qk", bufs=cfg["qk_bufs"],
                             name=f"qk{b}_{m}")
                for ch in range(NCH):
                    mm = ps.tile([128, 512], F32, tag="mm", bufs=cfg["mm_bufs"],
                                 name=f"mmq{b}_{m}_{ch}")
                    for k in range(CT):
                        nc.tensor.matmul(mm, wq[k][:, m * 128:(m + 1) * 128],
                                         xns[k][:, ch * 512:(ch + 1) * 512],
                                         start=(k == 0), stop=(k == CT - 1))
                    nc.vector.tensor_scalar_add(qt[:, ch * 512:(ch + 1) * 512],
                                                mm, bqkv_t[:, m:m + 1])
                return qt

            def vt_stile(b, st, xns):
                vt = sb.tile([128, C], F32R, tag="vt", bufs=cfg["vt_bufs"],
                             name=f"vt{b}_{st}")
                mm = ps.tile([128, 512], F32, tag="mm", bufs=cfg["mm_bufs"],
                             name=f"mmv{b}_{st}")
                for k in range(CT):
                    nc.tensor.matmul(mm, xns[k][:, st * 128:(st + 1) * 128],
                                     wq[k][:, 2 * C:3 * C],
                                     start=(k == 0), stop=(k == CT - 1))
                if use_v_bias:
                    nc.vector.scalar_tensor_tensor(out=vt, in0=mm, scalar=0.0,
                                                   in1=bv_bc, op0=OP.add, op1=OP.add)
                else:
                    nc.vector.tensor_copy(out=vt, in_=mm)
                return vt

            def alloc_on(b):
                return [sb.tile([128, S], F32R, tag="on", bufs=cfg["on_bufs"],
                                name=f"on{b}_{h}") for h in range(NH)]

            def attention_head(b, h, q_t, k_t, vts, on):
                for ch in range(NCH):
                    ets = []
                    for st in range(ST):
                        sc = ps.tile([128, 512], F32, tag="sc", bufs=cfg["sc_bufs"],
                                     name=f"sc{b}_{h}_{ch}_{st}")
                        nc.tensor.matmul(sc, k_t[:, st * 128:(st + 1) * 128],
                                         q_t[:, ch * 512:(ch + 1) * 512],
                                         start=True, stop=True)
                        et = sb.tile([128, 512], F32R, tag="et", bufs=cfg["et_bufs"],
                                     name=f"et{b}_{h}_{ch}_{st}")
                        nc.scalar.activation(out=et, in_=sc, func=ACT.Exp, scale=SCALE)
                        ets.append(et)
                    row = ps.tile([1, 512], F32, tag="row", bufs=1,
                                  name=f"row{b}_{h}_{ch}")
                    for st in range(ST):
                        nc.tensor.matmul(row, ones_t, ets[st],
                                         start=(st == 0), stop=(st == ST - 1))
                    rcp = sb.tile([1, 512], F32, tag="rcp", bufs=2,
                                  name=f"rcp{b}_{h}_{ch}")
                    nc.vector.reciprocal(out=rcp, in_=row)
                    nc.sync.dma_start(out=scr_rcp[b, h, ch], in_=rcp)
                    rbc = sb.tile([128, 512], F32, tag="rbc", bufs=2,
                                  name=f"rbc{b}_{h}_{ch}")
                    nc.sync.dma_start(
                        out=rbc,
                        in_=scr_rcp[b, h, ch].rearrange("(o s) -> o s", o=1)
                        .partition_broadcast(128))
                    av = ps.tile([128, 512], F32, tag="av", bufs=cfg["av_bufs"],
                                 name=f"av{b}_{h}_{ch}")
                    for st in range(ST):
                        nc.tensor.matmul(av, vts[st][:, h * HD:(h + 1) * HD], ets[st],
                                         start=(st == 0), stop=(st == ST - 1))
                    nc.vector.tensor_tensor(out=on[h][:, ch * 512:(ch + 1) * 512],
                                            in0=av, in1=rbc, op=OP.mult)

            def outproj(b, on):
                for m in range(CT):
                    rx = sb.tile([128, S], F32, tag="rx", bufs=cfg["rx_bufs"],
                                 name=f"rx{b}_{m}")
                    nc.sync.dma_start(out=rx, in_=x_d[b, m * 128:(m + 1) * 128, :])
                    res = sb.tile([128, S], F32, tag="res", bufs=cfg["res_bufs"],
                                  name=f"res{b}_{m}")
                    for ch in range(NCH):
                        mm = ps.tile([128, 512], F32, tag="mm", bufs=cfg["mm_bufs"],
                                     name=f"mmo{b}_{m}_{ch}")
                        for k in range(CT):
                            nc.tensor.matmul(mm, wo[k][:, m * 128:(m + 1) * 128],
                                             on[k][:, ch * 512:(ch + 1) * 512],
                                             start=(k == 0), stop=(k == CT - 1))
                        nc.vector.scalar_tensor_tensor(
                            out=res[:, ch * 512:(ch + 1) * 512], in0=mm,
                            scalar=bout_t[:, m:m + 1],
                            in1=rx[:, ch * 512:(ch + 1) * 512],
                            op0=OP.add, op1=OP.add)
                    nc.sync.dma_start(out=y_d[b, m * 128:(m + 1) * 128, :], in_=res)

            # batch 0 front
            xns0 = stats_and_norm(0)
            qk0 = {}
            for m in (0, 4, 1, 5, 2, 6, 3, 7):
                qk0[m] = qkv_mtile(0, m, xns0)
            vts0 = [vt_stile(0, st, xns0) for st in range(ST)]
            on0 = alloc_on(0)
            # attention(0) with batch-1 prep interleaved between heads
            attention_head(0, 0, qk0[0], qk0[4], vts0, on0)
            xns1 = stats_and_norm(1)
            attention_head(0, 1, qk0[1], qk0[5], vts0, on0)
            qk1 = {}
            qk1[0] = qkv_mtile(1, 0, xns1)
            qk1[4] = qkv_mtile(1, 4, xns1)
            attention_head(0, 2, qk0[2], qk0[6], vts0, on0)
            for m in (1, 5, 2, 6):
                qk1[m] = qkv_mtile(1, m, xns1)
            attention_head(0, 3, qk0[3], qk0[7], vts0, on0)
            for m in (3, 7):
                qk1[m] = qkv_mtile(1, m, xns1)
            vts1 = [vt_stile(1, st, xns1) for st in range(ST)]
            outproj(0, on0)
            on1 = alloc_on(1)
            for h in range(NH):
                attention_head(1, h, qk1[h], qk1[NH + h], vts1, on1)
            outproj(1, on1)
    nc.finalize()
    return nc


_cached = {}


def _get_program(use_v_bias: bool) -> bass.Bass:
    if use_v_bias not in _cached:
        _cached[use_v_bias] = build_program_v3(use_v_bias)
    return _cached[use_v_bias]


def kernel(x, gn_weight, gn_bias, qkv_w, qkv_b, out_w, out_b):
    x = np.ascontiguousarray(np.asarray(x, dtype=np.float32))
    gn_weight = np.asarray(gn_weight, dtype=np.float32)
    gn_bias = np.asarray(gn_bias, dtype=np.float32)
    qkv_w = np.asarray(qkv_w, dtype=np.float32)
    qkv_b = np.asarray(qkv_b, dtype=np.float32)
    out_w = np.asarray(out_w, dtype=np.float32)
    out_b = np.asarray(out_b, dtype=np.float32)

    # fold the GroupNorm affine into the QKV projection (host-side prep)
    w_eff = qkv_w * gn_weight[None, :]
    b_eff = qkv_b + qkv_w @ gn_bias
    wqkvT = np.ascontiguousarray(w_eff.T)            # [C, 3C]
    woutT = np.ascontiguousarray(out_w.T)            # [C, C]
    use_v_bias = bool(np.any(b_eff[2 * C:] != 0.0))

    nc = _get_program(use_v_bias)
    xs = x.reshape(B, C, S)
    in_maps = []
    for c in range(N_CORES):
        in_maps.append({
            "x": np.ascontiguousarray(xs[c * BPC:(c + 1) * BPC]),
            "wqkvT": wqkvT,
            "bqkv": np.ascontiguousarray(b_eff),
            "woutT": woutT,
            "bout": np.ascontiguousarray(out_b),
        })
    r = run_bass_kernel_spmd(nc, in_maps, list(range(N_CORES)))
    out = np.concatenate([r.results[c]["y"] for c in range(N_CORES)], axis=0)
    return out.reshape(B, C, H, W).astype(np.float32)


# revision 26
# speedup vs baseline: 306.3349x; 1.0513x over previous
"""AttentionBlock (GroupNorm -> MHA -> out-proj -> residual) on 8 TRN2 NeuronCores.

Problem: x (16, 512, 32, 32) fp32; GroupNorm(groups=1) over (C,H,W); spatial
flattened to a 1024-token sequence; 4 heads x 128 dim self-attention; output
projection; residual add.

Sharding: pure data-parallel over batch - 2 batch elements per core, no
collectives. Each core runs the identical program on its own x shard.

Layout strategy (per batch element, everything channel-major [c, s] so the PE
contracts over partitions with zero transposes):
  - GroupNorm stats via ACT Square+accum / DVE row-reduce, cross-partition via
    a ones-vector matmul; rstd by Newton rsqrt on DVE (avoids ACT table
    switch away from the Exp set). Partition broadcasts are 0-stride-AP DMAs.
  - QKV: qkv_cs[m, s] = qkv_wT.T @ x_norm  (Q, K stay [c, s]); V is computed
    directly transposed, vT[s, vd] = x_norm.T @ qkv_wT_v, so attention needs
    no transposes at all.
  - scoresT[s2, s1] = K.T @ Q per head; exp on ACT (PSUM -> SBUF, fused
    1/sqrt(hd) scale; softmax max-subtraction skipped - scores are O(1) by
    construction so exp cannot overflow).
  - row sums of exp via ones-vector matmul (PSUM accumulation over s2 tiles);
    normalization applied to the (small) AV output, with the reciprocal
    broadcast across partitions via a DRAM-bounce 0-stride DMA.
  - out-proj + residual fused into the PSUM-evacuation op on DVE.
  - All matmuls run in float32r (rounded-fp32, full PE rate at N=512;
    measured ~7.6e-4 mean rel err per K=128 dot, end-to-end 2.8e-5).

The shipped program is build_program_v3: the Tile scheduler assigns pool
slots in emission order, so batch-1 stats/QKV are emitted interleaved
between batch-0 attention heads to fill the PE gaps left by ACT exp pacing,
and the GroupNorm scalar chain is replicated across partitions via a K=1
ones outer-product instead of a serial DRAM bounce
(cost-model end-to-end: 210us -> 185us per core).

GroupNorm's affine (gn_weight/gn_bias) is folded into the QKV weights on the
host: qkv = W @ (xn * g + b) = (W * g) @ xn + (qkv_b + W @ b).
"""
import sys

sys.path.insert(0, "/opt/trn_rl_repo")

import numpy as np

import concourse.bass as bass
import concourse.mybir as mybir
import concourse.tile as tile
from concourse import bacc
from concourse.bass_utils import run_bass_kernel_spmd

F32 = mybir.dt.float32
F32R = mybir.dt.float32r
AX = mybir.AxisListType
OP = mybir.AluOpType
ACT = mybir.ActivationFunctionType

N_CORES = 8
B, C, H, W = 16, 512, 32, 32
S = H * W                     # 1024 sequence positions
NH, HD = 4, C // 4            # 4 heads x 128
BPC = B // N_CORES            # 2 batch elements per core
CT = C // 128                 # 4 channel tiles
ST = S // 128                 # 8 sequence tiles
NCH = S // 512                # 2 free-dim chunks of 512
EPS = 1e-5
SCALE = 1.0 / float(np.sqrt(HD))
N_ELEM = float(C * S)


DEFAULT_CFG = {
    "sc_bufs": 2, "av_bufs": 2, "mm_bufs": 3, "et_bufs": 12,
    "xn_bufs": 4, "qk_bufs": 8, "vt_bufs": 8, "on_bufs": 4,
    "xload_bufs": 5, "res_bufs": 2, "rx_bufs": 2,
    # debug/timing-shape flags (change semantics; model experiments only)
    "skip_norm": False, "exp_on_dve": False,
}


def build_program(use_v_bias: bool, cfg: dict | None = None) -> bass.Bass:
    cfg = {**DEFAULT_CFG, **(cfg or {})}
    nc = bacc.Bacc()
    x_d = nc.dram_tensor("x", [BPC, C, S], F32, kind="ExternalInput")
    wqkv_d = nc.dram_tensor("wqkvT", [C, 3 * C], F32, kind="ExternalInput")
    bqkv_d = nc.dram_tensor("bqkv", [3 * C], F32, kind="ExternalInput")
    wout_d = nc.dram_tensor("woutT", [C, C], F32, kind="ExternalInput")
    bout_d = nc.dram_tensor("bout", [C], F32, kind="ExternalInput")
    y_d = nc.dram_tensor("y", [BPC, C, S], F32, kind="ExternalOutput")
    # DRAM scratch for partition broadcasts (SBUF->DRAM->0-stride-read-back)
    scr_ms = nc.dram_tensor("scr_ms", [BPC, 2], F32)
    scr_rcp = nc.dram_tensor("scr_rcp", [BPC, NH, NCH, 512], F32)

    with tile.TileContext(nc) as tc:
        with (
            tc.tile_pool(name="const", bufs=1) as cpool,
            tc.tile_pool(name="sb", bufs=1) as sb,
            tc.tile_pool(name="ps", bufs=1, space="PSUM") as ps,
        ):
            # ---- constants ----
            wq = []
            for k in range(CT):
                t = cpool.tile([128, 3 * C], F32R, name=f"wq{k}")
                nc.gpsimd.dma_start(out=t, in_=wqkv_d[k * 128:(k + 1) * 128, :])
                wq.append(t)
            wo = []
            for k in range(CT):
                t = cpool.tile([128, C], F32R, name=f"wo{k}")
                nc.gpsimd.dma_start(out=t, in_=wout_d[k * 128:(k + 1) * 128, :])
                wo.append(t)
            bqkv_t = cpool.tile([128, 12], F32, name="bqkv_t")
            nc.sync.dma_start(out=bqkv_t, in_=bqkv_d[:].rearrange("(m p) -> p m", p=128))
            bout_t = cpool.tile([128, CT], F32, name="bout_t")
            nc.sync.dma_start(out=bout_t, in_=bout_d[:].rearrange("(m p) -> p m", p=128))
            ones32 = cpool.tile([128, 1], F32, name="ones32")
            nc.vector.memset(ones32, 1.0)
            ones_t = cpool.tile([128, 1], F32R, name="ones_t")
            nc.vector.tensor_copy(out=ones_t, in_=ones32)
            if use_v_bias:
                bv_bc = cpool.tile([128, C], F32, name="bv_bc")
                nc.sync.dma_start(
                    out=bv_bc,
                    in_=bqkv_d[2 * C:3 * C].rearrange("(o s) -> o s", o=1)
                    .partition_broadcast(128))

            for b in range(BPC):
                # ---- GroupNorm statistics ----
                xts = []
                partials = sb.tile([128, 2 * CT], F32, tag="part", bufs=2, name=f"part{b}")
                for t in range(CT):
                    xt = sb.tile([128, S], F32, tag="xload", bufs=cfg["xload_bufs"], name=f"x{b}_{t}")
                    nc.sync.dma_start(out=xt, in_=x_d[b, t * 128:(t + 1) * 128, :])
                    xts.append(xt)
                    sq = sb.tile([128, S], F32, tag="sqscr", bufs=2, name=f"sq{b}_{t}")
                    nc.scalar.activation(out=sq, in_=xt, func=ACT.Square,
                                         accum_out=partials[:, CT + t:CT + t + 1])
                    nc.vector.reduce_sum(out=partials[:, t:t + 1], in_=xt, axis=AX.X)
                partials_r = sb.tile([128, 2 * CT], F32R, tag="partr", bufs=2,
                                     name=f"partr{b}")
                nc.vector.tensor_copy(out=partials_r, in_=partials)
                stat_ps = ps.tile([1, 512], F32, tag="row", bufs=1, name=f"stat{b}")
                nc.tensor.matmul(stat_ps[0:1, 0:2 * CT], ones_t, partials_r,
                                 start=True, stop=True)
                # scalar chain on partition 0; cols: 0=mean 1=y(rstd) 2=v 3,4=tmp
                scal = sb.tile([1, 5], F32, tag="scal", bufs=2, name=f"scal{b}")
                nc.vector.reduce_sum(out=scal[:, 3:4], in_=stat_ps[0:1, 0:CT], axis=AX.X)
                nc.vector.reduce_sum(out=scal[:, 4:5], in_=stat_ps[0:1, CT:2 * CT], axis=AX.X)
                nc.vector.tensor_scalar_mul(scal[:, 0:1], scal[:, 3:4], 1.0 / N_ELEM)
                nc.vector.tensor_scalar_mul(scal[:, 4:5], scal[:, 4:5], 1.0 / N_ELEM)
                nc.vector.tensor_tensor(out=scal[:, 3:4], in0=scal[:, 0:1],
                                        in1=scal[:, 0:1], op=OP.mult)
                nc.vector.tensor_tensor(out=scal[:, 2:3], in0=scal[:, 4:5],
                                        in1=scal[:, 3:4], op=OP.subtract)
                nc.vector.tensor_scalar_add(scal[:, 2:3], scal[:, 2:3], EPS)
                # Newton rsqrt: y0 = 1/v, y <- y*(1.5 - 0.5*v*y^2), 3 iters
                nc.vector.reciprocal(out=scal[:, 1:2], in_=scal[:, 2:3])
                for _ in range(3):
                    nc.vector.tensor_tensor(out=scal[:, 3:4], in0=scal[:, 1:2],
                                            in1=scal[:, 1:2], op=OP.mult)
                    nc.vector.tensor_tensor(out=scal[:, 3:4], in0=scal[:, 3:4],
                                            in1=scal[:, 2:3], op=OP.mult)
                    nc.vector.tensor_scalar(scal[:, 3:4], scal[:, 3:4], -0.5, 1.5,
                                            op0=OP.mult, op1=OP.add)
                    nc.vector.tensor_tensor(out=scal[:, 1:2], in0=scal[:, 1:2],
                                            in1=scal[:, 3:4], op=OP.mult)
                # broadcast (mean, rstd) to all partitions via a DRAM bounce
                nc.sync.dma_start(out=scr_ms[b], in_=scal[0:1, 0:2])
                mbc = sb.tile([128, 2], F32, tag="mbc", bufs=2, name=f"mbc{b}")
                nc.sync.dma_start(
                    out=mbc,
                    in_=scr_ms[b].rearrange("(o s) -> o s", o=1).partition_broadcast(128))

                # ---- x_norm = (x - mean) * rstd, written as float32r ----
                xns = []
                for t in range(CT):
                    xn = sb.tile([128, S], F32R, tag="xn", bufs=cfg["xn_bufs"], name=f"xn{b}_{t}")
                    nc.vector.tensor_scalar(xn, xts[t], mbc[:, 0:1], mbc[:, 1:2],
                                            op0=OP.subtract, op1=OP.mult)
                    xns.append(xn)

                # ---- QKV projections: Q,K channel-major; V sequence-major ----
                qk = {}
                for m in (0, 4, 1, 5, 2, 6, 3, 7):
                    qt = sb.tile([128, S], F32R, tag="qk", bufs=cfg["qk_bufs"], name=f"qk{b}_{m}")
                    for ch in range(NCH):
                        mm = ps.tile([128, 512], F32, tag="mm", bufs=cfg["mm_bufs"], name=f"mmq{b}_{m}_{ch}")
                        for k in range(CT):
                            nc.tensor.matmul(mm, wq[k][:, m * 128:(m + 1) * 128],
                                             xns[k][:, ch * 512:(ch + 1) * 512],
                                             start=(k == 0), stop=(k == CT - 1))
                        nc.vector.tensor_scalar_add(qt[:, ch * 512:(ch + 1) * 512], mm,
                                                    bqkv_t[:, m:m + 1])
                    qk[m] = qt
                vts = []
                for st in range(ST):
                    vt = sb.tile([128, C], F32R, tag="vt", bufs=cfg["vt_bufs"], name=f"vt{b}_{st}")
                    mm = ps.tile([128, 512], F32, tag="mm", bufs=cfg["mm_bufs"], name=f"mmv{b}_{st}")
                    for k in range(CT):
                        nc.tensor.matmul(mm, xns[k][:, st * 128:(st + 1) * 128],
                                         wq[k][:, 2 * C:3 * C],
                                         start=(k == 0), stop=(k == CT - 1))
                    if use_v_bias:
                        nc.vector.scalar_tensor_tensor(out=vt, in0=mm, scalar=0.0,
                                                       in1=bv_bc, op0=OP.add, op1=OP.add)
                    else:
                        nc.vector.tensor_copy(out=vt, in_=mm)
                    vts.append(vt)

                # ---- attention, head by head ----
                on = []
                for h in range(NH):
                    ot = sb.tile([128, S], F32R, tag="on", bufs=cfg["on_bufs"], name=f"on{b}_{h}")
                    on.append(ot)
                for h in range(NH):
                    q_t, k_t = qk[h], qk[NH + h]
                    for ch in range(NCH):
                        ets = []
                        for st in range(ST):
                            sc = ps.tile([128, 512], F32, tag="sc", bufs=cfg["sc_bufs"],
                                         name=f"sc{b}_{h}_{ch}_{st}")
                            nc.tensor.matmul(sc, k_t[:, st * 128:(st + 1) * 128],
                                             q_t[:, ch * 512:(ch + 1) * 512],
                                             start=True, stop=True)
                            et = sb.tile([128, 512], F32R, tag="et", bufs=cfg["et_bufs"],
                                         name=f"et{b}_{h}_{ch}_{st}")
                            if cfg["exp_on_dve"]:
                                nc.vector.tensor_copy(out=et, in_=sc)
                            else:
                                nc.scalar.activation(out=et, in_=sc, func=ACT.Exp, scale=SCALE)
                            ets.append(et)
                        if not cfg["skip_norm"]:
                            row = ps.tile([1, 512], F32, tag="row", bufs=1,
                                          name=f"row{b}_{h}_{ch}")
                            for st in range(ST):
                                nc.tensor.matmul(row, ones_t, ets[st],
                                                 start=(st == 0), stop=(st == ST - 1))
                            rcp = sb.tile([1, 512], F32, tag="rcp", bufs=2,
                                          name=f"rcp{b}_{h}_{ch}")
                            nc.vector.reciprocal(out=rcp, in_=row)
                            nc.sync.dma_start(out=scr_rcp[b, h, ch], in_=rcp)
                            rbc = sb.tile([128, 512], F32, tag="rbc", bufs=2,
                                          name=f"rbc{b}_{h}_{ch}")
                            nc.sync.dma_start(
                                out=rbc,
                                in_=scr_rcp[b, h, ch].rearrange("(o s) -> o s", o=1)
                                .partition_broadcast(128))
                        av = ps.tile([128, 512], F32, tag="av", bufs=cfg["av_bufs"],
                                     name=f"av{b}_{h}_{ch}")
                        for st in range(ST):
                            nc.tensor.matmul(av, vts[st][:, h * HD:(h + 1) * HD], ets[st],
                                             start=(st == 0), stop=(st == ST - 1))
                        if cfg["skip_norm"]:
                            nc.vector.tensor_copy(
                                out=on[h][:, ch * 512:(ch + 1) * 512], in_=av)
                        else:
                            nc.vector.tensor_tensor(out=on[h][:, ch * 512:(ch + 1) * 512],
                                                    in0=av, in1=rbc, op=OP.mult)

                # ---- output projection + residual ----
                for m in range(CT):
                    rx = sb.tile([128, S], F32, tag="rx", bufs=cfg["rx_bufs"], name=f"rx{b}_{m}")
                    nc.sync.dma_start(out=rx, in_=x_d[b, m * 128:(m + 1) * 128, :])
                    res = sb.tile([128, S], F32, tag="res", bufs=cfg["res_bufs"], name=f"res{b}_{m}")
                    for ch in range(NCH):
                        mm = ps.tile([128, 512], F32, tag="mm", bufs=cfg["mm_bufs"],
                                     name=f"mmo{b}_{m}_{ch}")
                        for k in range(CT):
                            nc.tensor.matmul(mm, wo[k][:, m * 128:(m + 1) * 128],
                                             on[k][:, ch * 512:(ch + 1) * 512],
                                             start=(k == 0), stop=(k == CT - 1))
                        nc.vector.scalar_tensor_tensor(
                            out=res[:, ch * 512:(ch + 1) * 512], in0=mm,
                            scalar=bout_t[:, m:m + 1],
                            in1=rx[:, ch * 512:(ch + 1) * 512],
                            op0=OP.add, op1=OP.add)
                    nc.sync.dma_start(out=y_d[b, m * 128:(m + 1) * 128, :], in_=res)
    nc.finalize()
    return nc


def build_program_v2(use_v_bias: bool, cfg: dict | None = None) -> bass.Bass:
    """Phased emission: stats(b1) overlaps QKV(b0) (ACT is idle there), QKV(b1)
    fills PE gaps of attention(b0), and exp runs on [128, 1024] PSUM reads
    (halves ACT per-instr overhead). PSUM banks: sc 1x2 + av 2 + mm 3 + row 1 = 8.
    """
    cfg = {**DEFAULT_CFG, "xn_bufs": 8, "et_bufs": 8, "res_bufs": 1,
           "sqscr_bufs": 1, "xload_bufs": 4, "rx_bufs": 1, **(cfg or {})}
    nc = bacc.Bacc()
    x_d = nc.dram_tensor("x", [BPC, C, S], F32, kind="ExternalInput")
    wqkv_d = nc.dram_tensor("wqkvT", [C, 3 * C], F32, kind="ExternalInput")
    bqkv_d = nc.dram_tensor("bqkv", [3 * C], F32, kind="ExternalInput")
    wout_d = nc.dram_tensor("woutT", [C, C], F32, kind="ExternalInput")
    bout_d = nc.dram_tensor("bout", [C], F32, kind="ExternalInput")
    y_d = nc.dram_tensor("y", [BPC, C, S], F32, kind="ExternalOutput")
    scr_ms = nc.dram_tensor("scr_ms", [BPC, 2], F32)
    scr_rcp = nc.dram_tensor("scr_rcp", [BPC, NH, NCH, 512], F32)

    with tile.TileContext(nc) as tc:
        with (
            tc.tile_pool(name="const", bufs=1) as cpool,
            tc.tile_pool(name="sb", bufs=1) as sb,
            tc.tile_pool(name="ps", bufs=1, space="PSUM") as ps,
        ):
            wq = []
            for k in range(CT):
                t = cpool.tile([128, 3 * C], F32R, name=f"wq{k}")
                nc.gpsimd.dma_start(out=t, in_=wqkv_d[k * 128:(k + 1) * 128, :])
                wq.append(t)
            wo = []
            for k in range(CT):
                t = cpool.tile([128, C], F32R, name=f"wo{k}")
                nc.gpsimd.dma_start(out=t, in_=wout_d[k * 128:(k + 1) * 128, :])
                wo.append(t)
            bqkv_t = cpool.tile([128, 12], F32, name="bqkv_t")
            nc.sync.dma_start(out=bqkv_t, in_=bqkv_d[:].rearrange("(m p) -> p m", p=128))
            bout_t = cpool.tile([128, CT], F32, name="bout_t")
            nc.sync.dma_start(out=bout_t, in_=bout_d[:].rearrange("(m p) -> p m", p=128))
            ones32 = cpool.tile([128, 1], F32, name="ones32")
            nc.vector.memset(ones32, 1.0)
            ones_t = cpool.tile([128, 1], F32R, name="ones_t")
            nc.vector.tensor_copy(out=ones_t, in_=ones32)
            if use_v_bias:
                bv_bc = cpool.tile([128, C], F32, name="bv_bc")
                nc.sync.dma_start(
                    out=bv_bc,
                    in_=bqkv_d[2 * C:3 * C].rearrange("(o s) -> o s", o=1)
                    .partition_broadcast(128))

            def stats_and_norm(b):
                """Load x(b), compute mean/rstd, write x_norm(b) in f32r."""
                xts = []
                partials = sb.tile([128, 2 * CT], F32, tag="part", bufs=2,
                                   name=f"part{b}")
                for t in range(CT):
                    xt = sb.tile([128, S], F32, tag="xload",
                                 bufs=cfg["xload_bufs"], name=f"x{b}_{t}")
                    nc.sync.dma_start(out=xt, in_=x_d[b, t * 128:(t + 1) * 128, :])
                    xts.append(xt)
                    sq = sb.tile([128, S], F32, tag="sqscr",
                                 bufs=cfg["sqscr_bufs"], name=f"sq{b}_{t}")
                    nc.scalar.activation(out=sq, in_=xt, func=ACT.Square,
                                         accum_out=partials[:, CT + t:CT + t + 1])
                    nc.vector.reduce_sum(out=partials[:, t:t + 1], in_=xt, axis=AX.X)
                partials_r = sb.tile([128, 2 * CT], F32R, tag="partr", bufs=2,
                                     name=f"partr{b}")
                nc.vector.tensor_copy(out=partials_r, in_=partials)
                stat_ps = ps.tile([1, 512], F32, tag="row", bufs=1, name=f"stat{b}")
                nc.tensor.matmul(stat_ps[0:1, 0:2 * CT], ones_t, partials_r,
                                 start=True, stop=True)
                scal = sb.tile([1, 5], F32, tag="scal", bufs=2, name=f"scal{b}")
                nc.vector.reduce_sum(out=scal[:, 3:4], in_=stat_ps[0:1, 0:CT], axis=AX.X)
                nc.vector.reduce_sum(out=scal[:, 4:5], in_=stat_ps[0:1, CT:2 * CT],
                                     axis=AX.X)
                nc.vector.tensor_scalar_mul(scal[:, 0:1], scal[:, 3:4], 1.0 / N_ELEM)
                nc.vector.tensor_scalar_mul(scal[:, 4:5], scal[:, 4:5], 1.0 / N_ELEM)
                nc.vector.tensor_tensor(out=scal[:, 3:4], in0=scal[:, 0:1],
                                        in1=scal[:, 0:1], op=OP.mult)
                nc.vector.tensor_tensor(out=scal[:, 2:3], in0=scal[:, 4:5],
                                        in1=scal[:, 3:4], op=OP.subtract)
                nc.vector.tensor_scalar_add(scal[:, 2:3], scal[:, 2:3], EPS)
                nc.vector.reciprocal(out=scal[:, 1:2], in_=scal[:, 2:3])
                for _ in range(3):
                    nc.vector.tensor_tensor(out=scal[:, 3:4], in0=scal[:, 1:2],
                                            in1=scal[:, 1:2], op=OP.mult)
                    nc.vector.tensor_tensor(out=scal[:, 3:4], in0=scal[:, 3:4],
                                            in1=scal[:, 2:3], op=OP.mult)
                    nc.vector.tensor_scalar(scal[:, 3:4], scal[:, 3:4], -0.5, 1.5,
                                            op0=OP.mult, op1=OP.add)
                    nc.vector.tensor_tensor(out=scal[:, 1:2], in0=scal[:, 1:2],
                                            in1=scal[:, 3:4], op=OP.mult)
                nc.sync.dma_start(out=scr_ms[b], in_=scal[0:1, 0:2])
                mbc = sb.tile([128, 2], F32, tag="mbc", bufs=2, name=f"mbc{b}")
                nc.sync.dma_start(
                    out=mbc,
                    in_=scr_ms[b].rearrange("(o s) -> o s", o=1).partition_broadcast(128))
                xns = []
                for t in range(CT):
                    xn = sb.tile([128, S], F32R, tag="xn", bufs=cfg["xn_bufs"],
                                 name=f"xn{b}_{t}")
                    nc.vector.tensor_scalar(xn, xts[t], mbc[:, 0:1], mbc[:, 1:2],
                                            op0=OP.subtract, op1=OP.mult)
                    xns.append(xn)
                return xns

            def qkv(b, xns):
                qk = {}
                for m in (0, 4, 1, 5, 2, 6, 3, 7):
                    qt = sb.tile([128, S], F32R, tag="qk", bufs=cfg["qk_bufs"],
                                 name=f"qk{b}_{m}")
                    for ch in range(NCH):
                        mm = ps.tile([128, 512], F32, tag="mm", bufs=cfg["mm_bufs"],
                                     name=f"mmq{b}_{m}_{ch}")
                        for k in range(CT):
                            nc.tensor.matmul(mm, wq[k][:, m * 128:(m + 1) * 128],
                                             xns[k][:, ch * 512:(ch + 1) * 512],
                                             start=(k == 0), stop=(k == CT - 1))
                        nc.vector.tensor_scalar_add(qt[:, ch * 512:(ch + 1) * 512],
                                                    mm, bqkv_t[:, m:m + 1])
                    qk[m] = qt
                vts = []
                for st in range(ST):
                    vt = sb.tile([128, C], F32R, tag="vt", bufs=cfg["vt_bufs"],
                                 name=f"vt{b}_{st}")
                    mm = ps.tile([128, 512], F32, tag="mm", bufs=cfg["mm_bufs"],
                                 name=f"mmv{b}_{st}")
                    for k in range(CT):
                        nc.tensor.matmul(mm, xns[k][:, st * 128:(st + 1) * 128],
                                         wq[k][:, 2 * C:3 * C],
                                         start=(k == 0), stop=(k == CT - 1))
                    if use_v_bias:
                        nc.vector.scalar_tensor_tensor(out=vt, in0=mm, scalar=0.0,
                                                       in1=bv_bc, op0=OP.add, op1=OP.add)
                    else:
                        nc.vector.tensor_copy(out=vt, in_=mm)
                    vts.append(vt)
                return qk, vts

            def attention(b, qk, vts):
                on = []
                for h in range(NH):
                    ot = sb.tile([128, S], F32R, tag="on", bufs=cfg["on_bufs"],
                                 name=f"on{b}_{h}")
                    on.append(ot)
                for h in range(NH):
                    q_t, k_t = qk[h], qk[NH + h]
                    ets = []
                    for st in range(ST):
                        sc = ps.tile([128, S], F32, tag="sc", bufs=1,
                                     name=f"sc{b}_{h}_{st}")
                        for ch in range(NCH):
                            nc.tensor.matmul(sc[:, ch * 512:(ch + 1) * 512],
                                             k_t[:, st * 128:(st + 1) * 128],
                                             q_t[:, ch * 512:(ch + 1) * 512],
                                             start=True, stop=True)
                        et = sb.tile([128, S], F32R, tag="et", bufs=cfg["et_bufs"],
                                     name=f"et{b}_{h}_{st}")
                        nc.scalar.activation(out=et, in_=sc, func=ACT.Exp, scale=SCALE)
                        ets.append(et)
                    for ch in range(NCH):
                        chs = slice(ch * 512, (ch + 1) * 512)
                        row = ps.tile([1, 512], F32, tag="row", bufs=1,
                                      name=f"row{b}_{h}_{ch}")
                        for st in range(ST):
                            nc.tensor.matmul(row, ones_t, ets[st][:, chs],
                                             start=(st == 0), stop=(st == ST - 1))
                        rcp = sb.tile([1, 512], F32, tag="rcp", bufs=2,
                                      name=f"rcp{b}_{h}_{ch}")
                        nc.vector.reciprocal(out=rcp, in_=row)
                        nc.sync.dma_start(out=scr_rcp[b, h, ch], in_=rcp)
                        rbc = sb.tile([128, 512], F32, tag="rbc", bufs=2,
                                      name=f"rbc{b}_{h}_{ch}")
                        nc.sync.dma_start(
                            out=rbc,
                            in_=scr_rcp[b, h, ch].rearrange("(o s) -> o s", o=1)
                            .partition_broadcast(128))
                        av = ps.tile([128, 512], F32, tag="av", bufs=cfg["av_bufs"],
                                     name=f"av{b}_{h}_{ch}")
                        for st in range(ST):
                            nc.tensor.matmul(av, vts[st][:, h * HD:(h + 1) * HD],
                                             ets[st][:, chs],
                                             start=(st == 0), stop=(st == ST - 1))
                        nc.vector.tensor_tensor(out=on[h][:, chs], in0=av, in1=rbc,
                                                op=OP.mult)
                return on

            def outproj(b, on):
                for m in range(CT):
                    rx = sb.tile([128, S], F32, tag="rx", bufs=cfg["rx_bufs"],
                                 name=f"rx{b}_{m}")
                    nc.sync.dma_start(out=rx, in_=x_d[b, m * 128:(m + 1) * 128, :])
                    res = sb.tile([128, S], F32, tag="res", bufs=cfg["res_bufs"],
                                  name=f"res{b}_{m}")
                    for ch in range(NCH):
                        mm = ps.tile([128, 512], F32, tag="mm", bufs=cfg["mm_bufs"],
                                     name=f"mmo{b}_{m}_{ch}")
                        for k in range(CT):
                            nc.tensor.matmul(mm, wo[k][:, m * 128:(m + 1) * 128],
                                             on[k][:, ch * 512:(ch + 1) * 512],
                                             start=(k == 0), stop=(k == CT - 1))
                        nc.vector.scalar_tensor_tensor(
                            out=res[:, ch * 512:(ch + 1) * 512], in0=mm,
                            scalar=bout_t[:, m:m + 1],
                            in1=rx[:, ch * 512:(ch + 1) * 512],
                            op0=OP.add, op1=OP.add)
                    nc.sync.dma_start(out=y_d[b, m * 128:(m + 1) * 128, :], in_=res)

            xns0 = stats_and_norm(0)
            qk0, vts0 = qkv(0, xns0)
            xns1 = stats_and_norm(1)   # ACT/DVE overlap QKV(0) on PE
            on0 = attention(0, qk0, vts0)
            outproj(0, on0)
            qk1, vts1 = qkv(1, xns1)   # fills PE gaps during attention(0)
            on1 = attention(1, qk1, vts1)
            outproj(1, on1)
    nc.finalize()
    return nc


def build_program_v3(use_v_bias: bool, cfg: dict | None = None) -> bass.Bass:
    """v1 shapes ([128,512] exp, sc bufs 2) with fine-grained interleaved
    emission: the Tile scheduler allocates pool slots in emission order, so
    batch-1 stats/QKV are emitted BETWEEN batch-0 attention heads to fill the
    PE gaps that ACT exp pacing leaves.
    """
    cfg = {**DEFAULT_CFG, "xn_bufs": 8, "xload_bufs": 4, **(cfg or {})}
    nc = bacc.Bacc()
    x_d = nc.dram_tensor("x", [BPC, C, S], F32, kind="ExternalInput")
    wqkv_d = nc.dram_tensor("wqkvT", [C, 3 * C], F32, kind="ExternalInput")
    bqkv_d = nc.dram_tensor("bqkv", [3 * C], F32, kind="ExternalInput")
    wout_d = nc.dram_tensor("woutT", [C, C], F32, kind="ExternalInput")
    bout_d = nc.dram_tensor("bout", [C], F32, kind="ExternalInput")
    y_d = nc.dram_tensor("y", [BPC, C, S], F32, kind="ExternalOutput")
    scr_ms = nc.dram_tensor("scr_ms", [BPC, 2], F32)
    scr_rcp = nc.dram_tensor("scr_rcp", [BPC, NH, NCH, 512], F32)

    with tile.TileContext(nc) as tc:
        with (
            tc.tile_pool(name="const", bufs=1) as cpool,
            tc.tile_pool(name="sb", bufs=1) as sb,
            tc.tile_pool(name="ps", bufs=1, space="PSUM") as ps,
        ):
            wq = []
            for k in range(CT):
                t = cpool.tile([128, 3 * C], F32R, name=f"wq{k}")
                nc.gpsimd.dma_start(out=t, in_=wqkv_d[k * 128:(k + 1) * 128, :])
                wq.append(t)
            wo = []
            for k in range(CT):
                t = cpool.tile([128, C], F32R, name=f"wo{k}")
                nc.gpsimd.dma_start(out=t, in_=wout_d[k * 128:(k + 1) * 128, :])
                wo.append(t)
            bqkv_t = cpool.tile([128, 12], F32, name="bqkv_t")
            nc.sync.dma_start(out=bqkv_t, in_=bqkv_d[:].rearrange("(m p) -> p m", p=128))
            bout_t = cpool.tile([128, CT], F32, name="bout_t")
            nc.sync.dma_start(out=bout_t, in_=bout_d[:].rearrange("(m p) -> p m", p=128))
            ones32 = cpool.tile([128, 1], F32, name="ones32")
            nc.vector.memset(ones32, 1.0)
            ones_t = cpool.tile([128, 1], F32R, name="ones_t")
            nc.vector.tensor_copy(out=ones_t, in_=ones32)
            onesr32 = cpool.tile([1, 128], F32, name="onesr32")
            nc.vector.memset(onesr32, 1.0)
            ones_row = cpool.tile([1, 128], F32R, name="ones_row")
            nc.vector.tensor_copy(out=ones_row, in_=onesr32)
            if use_v_bias:
                bv_bc = cpool.tile([128, C], F32, name="bv_bc")
                nc.sync.dma_start(
                    out=bv_bc,
                    in_=bqkv_d[2 * C:3 * C].rearrange("(o s) -> o s", o=1)
                    .partition_broadcast(128))

            def stats_and_norm(b):
                xts = []
                partials = sb.tile([128, 2 * CT], F32, tag="part", bufs=2,
                                   name=f"part{b}")
                for t in range(CT):
                    xt = sb.tile([128, S], F32, tag="xload",
                                 bufs=cfg["xload_bufs"], name=f"x{b}_{t}")
                    nc.sync.dma_start(out=xt, in_=x_d[b, t * 128:(t + 1) * 128, :])
                    xts.append(xt)
                    sq = sb.tile([128, S], F32, tag="sqscr", bufs=1, name=f"sq{b}_{t}")
                    nc.scalar.activation(out=sq, in_=xt, func=ACT.Square,
                                         accum_out=partials[:, CT + t:CT + t + 1])
                    nc.vector.reduce_sum(out=partials[:, t:t + 1], in_=xt, axis=AX.X)
                partials_r = sb.tile([128, 2 * CT], F32R, tag="partr", bufs=2,
                                     name=f"partr{b}")
                nc.vector.tensor_copy(out=partials_r, in_=partials)
                stat_ps = ps.tile([1, 512], F32, tag="row", bufs=1, name=f"stat{b}")
                nc.tensor.matmul(stat_ps[0:1, 0:2 * CT], ones_t, partials_r,
                                 start=True, stop=True)
                # replicate the 8 partial sums to all 128 partitions with a
                # K=1 ones outer-product (no DRAM bounce on the critical path)
                stat_sb = sb.tile([1, 2 * CT], F32R, tag="statsb", bufs=2,
                                  name=f"statsb{b}")
                nc.vector.tensor_copy(out=stat_sb, in_=stat_ps[0:1, 0:2 * CT])
                bc_ps = ps.tile([128, 512], F32, tag="mm", bufs=cfg["mm_bufs"],
                                name=f"bcps{b}")
                nc.tensor.matmul(bc_ps[:, 0:2 * CT], ones_row, stat_sb,
                                 start=True, stop=True)
                # chain replicated across partitions; cols: 0=mean 1=y 2=v 3=tmp
                scal = sb.tile([128, 5], F32, tag="scal", bufs=2, name=f"scal{b}")
                nc.vector.reduce_sum(out=scal[:, 3:4], in_=bc_ps[:, 0:CT], axis=AX.X)
                nc.vector.reduce_sum(out=scal[:, 4:5], in_=bc_ps[:, CT:2 * CT],
                                     axis=AX.X)
                nc.vector.tensor_scalar_mul(scal[:, 0:1], scal[:, 3:4], 1.0 / N_ELEM)
                nc.vector.tensor_scalar_mul(scal[:, 4:5], scal[:, 4:5], 1.0 / N_ELEM)
                # v = -(mean*mean - ex2) + EPS = var + EPS
                nc.vector.scalar_tensor_tensor(out=scal[:, 2:3], in0=scal[:, 0:1],
                                               scalar=scal[:, 0:1], in1=scal[:, 4:5],
                                               op0=OP.mult, op1=OP.subtract)
                nc.vector.tensor_scalar(scal[:, 2:3], scal[:, 2:3], -1.0, EPS,
                                        op0=OP.mult, op1=OP.add)
                nc.vector.reciprocal(out=scal[:, 1:2], in_=scal[:, 2:3])
                for _ in range(3):
                    nc.vector.scalar_tensor_tensor(out=scal[:, 3:4], in0=scal[:, 1:2],
                                                   scalar=scal[:, 1:2],
                                                   in1=scal[:, 2:3],
                                                   op0=OP.mult, op1=OP.mult)
                    nc.vector.tensor_scalar(scal[:, 3:4], scal[:, 3:4], -0.5, 1.5,
                                            op0=OP.mult, op1=OP.add)
                    nc.vector.tensor_tensor(out=scal[:, 1:2], in0=scal[:, 1:2],
                                            in1=scal[:, 3:4], op=OP.mult)
                xns = []
                for t in range(CT):
                    xn = sb.tile([128, S], F32R, tag="xn", bufs=cfg["xn_bufs"],
                                 name=f"xn{b}_{t}")
                    nc.vector.tensor_scalar(xn, xts[t], scal[:, 0:1], scal[:, 1:2],
                                            op0=OP.subtract, op1=OP.mult)
                    xns.append(xn)
                return xns

            def qkv_mtile(b, m, xns):
                qt = sb.tile([128, S], F32R, tag="qk", bufs=cfg["qk_bufs"],
                             name=f"qk{b}_{m}")
                for ch in range(NCH):
                    mm = ps.tile([128, 512], F32, tag="mm", bufs=cfg["mm_bufs"],
                                 name=f"mmq{b}_{m}_{ch}")
                    for k in range(CT):
                        nc.tensor.matmul(mm, wq[k][:, m * 128:(m + 1) * 128],
                                         xns[k][:, ch * 512:(ch + 1) * 512],
                                         start=(k == 0), stop=(k == CT - 1))
                    nc.vector.tensor_scalar_add(qt[:, ch * 512:(ch + 1) * 512],
                                                mm, bqkv_t[:, m:m + 1])
                return qt

            def vt_stile(b, st, xns):
                vt = sb.tile([128, C], F32R, tag="vt", bufs=cfg["vt_bufs"],
                             name=f"vt{b}_{st}")
                mm = ps.tile([128, 512], F32, tag="mm", bufs=cfg["mm_bufs"],
                             name=f"mmv{b}_{st}")
                for k in range(CT):
                    nc.tensor.matmul(mm, xns[k][:, st * 128:(st + 1) * 128],
                                     wq[k][:, 2 * C:3 * C],
                                     start=(k == 0), stop=(k == CT - 1))
                if use_v_bias:
                    nc.vector.scalar_tensor_tensor(out=vt, in0=mm, scalar=0.0,
                                                   in1=bv_bc, op0=OP.add, op1=OP.add)
                else:
                    nc.vector.tensor_copy(out=vt, in_=mm)
                return vt

            def alloc_on(b):
                return [sb.tile([128, S], F32R, tag="on", bufs=cfg["on_bufs"],
                                name=f"on{b}_{h}") for h in range(NH)]

            def attention_head(b, h, q_t, k_t, vts, on):
                for ch in range(NCH):
                    ets = []
                    for st in range(ST):
                        sc = ps.tile([128, 512], F32, tag="sc", bufs=cfg["sc_bufs"],
                                     name=f"sc{b}_{h}_{ch}_{st}")
                        nc.tensor.matmul(sc, k_t[:, st * 128:(st + 1) * 128],
                                         q_t[:, ch * 512:(ch + 1) * 512],
                                         start=True, stop=True)
                        et = sb.tile([128, 512], F32R, tag="et", bufs=cfg["et_bufs"],
                                     name=f"et{b}_{h}_{ch}_{st}")
                        nc.scalar.activation(out=et, in_=sc, func=ACT.Exp, scale=SCALE)
                        ets.append(et)
                    row = ps.tile([1, 512], F32, tag="row", bufs=1,
                                  name=f"row{b}_{h}_{ch}")
                    for st in range(ST):
                        nc.tensor.matmul(row, ones_t, ets[st],
                                         start=(st == 0), stop=(st == ST - 1))
                    rcp = sb.tile([1, 512], F32, tag="rcp", bufs=2,
                                  name=f"rcp{b}_{h}_{ch}")
                    nc.vector.reciprocal(out=rcp, in_=row)
                    nc.sync.dma_start(out=scr_rcp[b, h, ch], in_=rcp)
                    rbc = sb.tile([128, 512], F32, tag="rbc", bufs=2,
                                  name=f"rbc{b}_{h}_{ch}")
                    nc.sync.dma_start(
                        out=rbc,
                        in_=scr_rcp[b, h, ch].rearrange("(o s) -> o s", o=1)
                        .partition_broadcast(128))
                    av = ps.tile([128, 512], F32, tag="av", bufs=cfg["av_bufs"],
                                 name=f"av{b}_{h}_{ch}")
                    for st in range(ST):
                        nc.tensor.matmul(av, vts[st][:, h * HD:(h + 1) * HD], ets[st],
                                         start=(st == 0), stop=(st == ST - 1))
                    nc.vector.tensor_tensor(out=on[h][:, ch * 512:(ch + 1) * 512],
                                            in0=av, in1=rbc, op=OP.mult)

            def outproj(b, on):
                for m in range(CT):
                    rx = sb.tile([128, S], F32, tag="rx", bufs=cfg["rx_bufs"],
                                 name=f"rx{b}_{m}")
                    nc.sync.dma_start(out=rx, in_=x_d[b, m * 128:(m + 1) * 128, :])
                    res = sb.tile([128, S], F32, tag="res", bufs=cfg["res_bufs"],
                                  name=f"res{b}_{m}")
                    for ch in range(NCH):
                        mm = ps.tile([128, 512], F32, tag="mm", bufs=cfg["mm_bufs"],
                                     name=f"mmo{b}_{m}_{ch}")
                        for k in range(CT):
                            nc.tensor.matmul(mm, wo[k][:, m * 128:(m + 1) * 128],
                                             on[k][:, ch * 512:(ch + 1) * 512],
                                             start=(k == 0), stop=(k == CT - 1))
                        nc.vector.scalar_tensor_tensor(
                            out=res[:, ch * 512:(ch + 1) * 512], in0=mm,
                            scalar=bout_t[:, m:m + 1],
                            in1=rx[:, ch * 512:(ch + 1) * 512],
                            op0=OP.add, op1=OP.add)
                    nc.sync.dma_start(out=y_d[b, m * 128:(m + 1) * 128, :], in_=res)

            # batch 0 front
            xns0 = stats_and_norm(0)
            qk0 = {}
            for m in (0, 4, 1, 5, 2, 6, 3, 7):
                qk0[m] = qkv_mtile(0, m, xns0)
            vts0 = [vt_stile(0, st, xns0) for st in range(ST)]
            on0 = alloc_on(0)
            # attention(0) with batch-1 prep interleaved between heads
            attention_head(0, 0, qk0[0], qk0[4], vts0, on0)
            xns1 = stats_and_norm(1)
            attention_head(0, 1, qk0[1], qk0[5], vts0, on0)
            qk1 = {}
            qk1[0] = qkv_mtile(1, 0, xns1)
            qk1[4] = qkv_mtile(1, 4, xns1)
            attention_head(0, 2, qk0[2], qk0[6], vts0, on0)
            for m in (1, 5, 2, 6):
                qk1[m] = qkv_mtile(1, m, xns1)
            attention_head(0, 3, qk0[3], qk0[7], vts0, on0)
            for m in (3, 7):
                qk1[m] = qkv_mtile(1, m, xns1)
            vts1 = [vt_stile(1, st, xns1) for st in range(ST)]
            outproj(0, on0)
            on1 = alloc_on(1)
            for h in range(NH):
                attention_head(1, h, qk1[h], qk1[NH + h], vts1, on1)
            outproj(1, on1)
    nc.finalize()
    return nc


_cached = {}


def _get_program(use_v_bias: bool) -> bass.Bass:
    if use_v_bias not in _cached:
        _cached[use_v_bias] = build_program_v3(use_v_bias)
    return _cached[use_v_bias]


def kernel(x, gn_weight, gn_bias, qkv_w, qkv_b, out_w, out_b):
    x = np.ascontiguousarray(np.asarray(x, dtype=np.float32))
    gn_weight = np.asarray(gn_weight, dtype=np.float32)
    gn_bias = np.asarray(gn_bias, dtype=np.float32)
    qkv_w = np.asarray(qkv_w, dtype=np.float32)
    qkv_b = np.asarray(qkv_b, dtype=np.float32)
    out_w = np.asarray(out_w, dtype=np.float32)
    out_b = np.asarray(out_b, dtype=np.float32)

    # fold the GroupNorm affine into the QKV projection (host-side prep)
    w_eff = qkv_w * gn_weight[None, :]
    b_eff = qkv_b + qkv_w @ gn_bias
    wqkvT = np.ascontiguousarray(w_eff.T)            # [C, 3C]
    woutT = np.ascontiguousarray(out_w.T)            # [C, C]
    use_v_bias = bool(np.any(b_eff[2 * C:] != 0.0))

    nc = _get_program(use_v_bias)
    xs = x.reshape(B, C, S)
    in_maps = []
    for c in range(N_CORES):
        in_maps.append({
            "x": np.ascontiguousarray(xs[c * BPC:(c + 1) * BPC]),
            "wqkvT": wqkvT,
            "bqkv": np.ascontiguousarray(b_eff),
            "woutT": woutT,
            "bout": np.ascontiguousarray(out_b),
        })
    r = run_bass_kernel_spmd(nc, in_maps, list(range(N_CORES)))
    out = np.concatenate([r.results[c]["y"] for c in range(N_CORES)], axis=0)
    return out.reshape(B, C, H, W).astype(np.float32)
